# revision 51
# baseline (speedup 1.0000x reference)
"""ALBERT attention layer (B=4, S=1024, D=1024, H=16) on 8 Trainium2 NeuronCores.

Sharding: core c = (batch b = c//2, query-half qh = c%2). Each core computes
the attention output rows q0 = qh*512 .. q0+512 of its batch: it projects
K/V for the full batch (duplicated across the 2 cores sharing a batch --
cheaper than a collective), Q for its own rows only, and produces its slice
of both outputs (out[b, q0:q0+512, :] and probs[b, :, q0:q0+512, :]).

Layout strategy per core:
  - hidden_states arrives host-pre-transposed (hsT, [D, S], bf16) so that
    K^T/Q^T ([d_col, s]) and V ([s, d_col]) all come straight out of the
    tensor engine with no on-device transposes.
  - scores are computed twice: once as S[q, k] (softmax over the free dim,
    exp with fused row-sum accum_out -> f32 probs output), and once as
    S^T[k, q] (row-tiled K=64 matmul pairs) whose exp gives the bf16
    operand P^T for the context matmul, which directly yields ctx^T --
    exactly the lhsT the output projection needs.
  - softmax 1/sum for the ctx path is transposed once per head-pair via the
    tensor engine ([128, 8] -> [8, 128]) and broadcast across partitions
    with a tiny DRAM round-trip.
  - head pairs are software-pipelined: the S-side (probs) work of pair hp
    is emitted alongside the S^T/ctx work of pair hp-1 so ScalarE (the
    long pole: 128 exp passes) always has runway and PE never idles long
    enough for the HAM clock gate to drop it to half rate.
"""

import numpy as np
import ml_dtypes

B, S, D, H = 4, 1024, 1024, 16
DH = D // H           # 64
P = 128
QL = S // 2           # 512 query rows per core
NCORES = 2 * B        # 8
DC = D // P           # 8 column chunks
QC = QL // P          # 4 query chunks per core
HP = H // 2           # 8 head pairs
SCALE = 1.0 / 32.0    # ALBERT: 1/sqrt(hidden_size) = 1/sqrt(1024)
EPS = 1e-12

_BUILD_CACHE = {}


def _split_wide_waits(nc, mybir, max_waits=1):
    """This walrus build rejects instructions carrying more than one sync
    wait (TPB_CTRL-class ops at least; the Tile kernel-tail drain aggregates
    one per busy proc). Move excess waits onto preceding same-engine NOPs --
    semantically identical since the engine executes its stream in order."""
    n_split = 0
    for f in nc.m.functions:
        for bb in f.blocks:
            insts = list(bb.instructions)
            out = []
            changed = False
            for inst in insts:
                si = getattr(inst, "sync_info", None)
                waits = list(si.on_wait) if (si is not None and si.on_wait) else []
                if len(waits) > max_waits:
                    n_split += 1
                    changed = True
                    k = 0
                    while len(waits) - k > max_waits:
                        chunk = waits[k:k + max_waits]
                        k += max_waits
                        nop = mybir.InstNoOp(
                            name=nc.get_next_instruction_name(),
                            sync_info=mybir.SyncInfo(on_wait=chunk, on_update=[]),
                            bass_nofuse=True,
                            engine=inst.engine,
                        )
                        nc.register_instruction(nop)
                        out.append(nop)
                    si.on_wait = waits[k:]
                out.append(inst)
            if changed:
                bb.instructions = out
    return n_split


def _build(use_bv, use_mask, use_gamma, use_beta):
    import concourse.bass as bass
    import concourse.mybir as mybir
    import concourse.tile as tile
    from concourse.masks import make_identity
    from contextlib import ExitStack

    f32 = mybir.dt.float32
    bf16 = mybir.dt.bfloat16
    AF = mybir.ActivationFunctionType
    ALU = mybir.AluOpType

    nc = bass.Bass("TRN2", target_bir_lowering=False, debug=False,
                   num_devices=NCORES)

    # ---- I/O ----
    hsT = nc.dram_tensor("hsT", [D, S], bf16, kind="ExternalInput")
    hsqT = nc.dram_tensor("hsqT", [D, QL], bf16, kind="ExternalInput")
    res = nc.dram_tensor("res", [QL, D], f32, kind="ExternalInput")
    wq = nc.dram_tensor("wq", [D, D], bf16, kind="ExternalInput")
    wk = nc.dram_tensor("wk", [D, D], bf16, kind="ExternalInput")
    wv = nc.dram_tensor("wv", [D, D], bf16, kind="ExternalInput")
    wo = nc.dram_tensor("wo", [D, D], bf16, kind="ExternalInput")
    bqr = nc.dram_tensor("bqr", [P, DC], f32, kind="ExternalInput")
    bkr = nc.dram_tensor("bkr", [P, DC], f32, kind="ExternalInput")
    if use_bv:
        bvrep = nc.dram_tensor("bvrep", [P, D], f32, kind="ExternalInput")
    if use_mask:
        # 32 * (-10000) * (1 - mask[k]): row-replicated for the S[q,k] psum
        # add, and per-partition columns for the S^T[k,q] psum add.
        mrep = nc.dram_tensor("mrep", [P, S], f32, kind="ExternalInput")
        mcol = nc.dram_tensor("mcol", [P, DC], f32, kind="ExternalInput")
    if use_gamma:
        grep = nc.dram_tensor("grep", [P, D], f32, kind="ExternalInput")
    if use_beta:
        brep = nc.dram_tensor("brep", [P, D], f32, kind="ExternalInput")
    probs_o = nc.dram_tensor("probs", [H, QL, S], f32, kind="ExternalOutput")
    out_o = nc.dram_tensor("out", [QL, D], f32, kind="ExternalOutput")

    with tile.TileContext(nc) as tc:
        with ExitStack() as ctx:
            singles = ctx.enter_context(tc.tile_pool(name="singles", bufs=1))
            probsp = ctx.enter_context(tc.tile_pool(name="probsp", bufs=4))
            # ptp/invdp live OUTSIDE the stageA region: phase-2 tiles must
            # not inherit released-zone deps on stage A's last weight reads
            # (that would serialize the phase transition and cool the PE).
            # Stage-C-only tensors (wo/res/x) go into the released region
            # instead -- their late deps are harmless.
            ptp = ctx.enter_context(tc.tile_pool(name="ptp", bufs=2))
            invdp = ctx.enter_context(tc.tile_pool(name="invdp", bufs=2))
            statp = ctx.enter_context(tc.tile_pool(name="statp", bufs=4))
            # PSUM budget (16KB/partition): proj 3x1 + sp 2x2 + cx 1x1 banks
            psP = ctx.enter_context(tc.tile_pool(name="psP", bufs=3, space="PSUM"))
            psS = ctx.enter_context(tc.tile_pool(name="psS", bufs=2, space="PSUM"))
            psC = ctx.enter_context(tc.tile_pool(name="psC", bufs=1, space="PSUM"))
            dramp = ctx.enter_context(tc.tile_pool(name="dramp", bufs=1, space="DRAM"))

            def load_chunks(pool, name, src, width, defer=False):
                """Allocate chunk tiles; if defer, return the DMA thunks so
                the caller can sequence loads in dependency-priority order."""
                tiles, thunks = [], []
                for dd in range(DC):
                    t = pool.tile([P, width], bf16, tag=f"{name}{dd}",
                                  name=f"{name}{dd}")
                    lo, hi = dd * P, (dd + 1) * P
                    thunks.append(
                        lambda t=t, lo=lo, hi=hi: nc.sync.dma_start(
                            out=t, in_=src[lo:hi, :]))
                    tiles.append(t)
                if not defer:
                    for th in thunks:
                        th()
                return (tiles, thunks) if defer else tiles

            bqr_sb = singles.tile([P, DC], f32, tag="bqr")
            nc.sync.dma_start(out=bqr_sb, in_=bqr[:, :])
            bkr_sb = singles.tile([P, DC], f32, tag="bkr")
            nc.sync.dma_start(out=bkr_sb, in_=bkr[:, :])
            if use_bv:
                bv_sb = singles.tile([P, D], f32, tag="bvrep")
                nc.sync.dma_start(out=bv_sb, in_=bvrep[:, :])
            if use_mask:
                mrep_sb = singles.tile([P, S], f32, tag="mrep")
                nc.sync.dma_start(out=mrep_sb, in_=mrep[:, :])
                mcol_sb = singles.tile([P, DC], f32, tag="mcol")
                nc.sync.dma_start(out=mcol_sb, in_=mcol[:, :])
            if use_gamma:
                g_sb = singles.tile([P, D], f32, tag="grep")
                nc.sync.dma_start(out=g_sb, in_=grep[:, :])
            if use_beta:
                be_sb = singles.tile([P, D], f32, tag="brep")
                nc.sync.dma_start(out=be_sb, in_=brep[:, :])

            ident = singles.tile([P, P], f32, tag="ident")
            make_identity(nc, ident)
            eps_t = singles.tile([P, 1], f32, tag="eps")
            nc.vector.memset(eps_t, EPS)

            # per-(head, q-chunk) exp row sums: column h*QC + qc
            d_all = singles.tile([P, H * QC], f32, tag="d_all")
            inv_all = singles.tile([P, H * QC], f32, tag="inv_all")
            invd_dram = dramp.tile([H * QC, P], f32)

            def s_side(hp):
                """S[q,k] -> exp/softmax -> probs rows; leaves 1/d (row
                layout) in invd_dram for st_ctx to broadcast later."""
                h0, h1 = 2 * hp, 2 * hp + 1
                r0, r1 = slice(0, DH), slice(DH, 2 * DH)
                for qc in range(QC):
                    qsl = slice(qc * P, (qc + 1) * P)
                    sp = [psS.tile([P, S], f32, tag="sp", name=f"sp{i}")
                          for i in range(2)]
                    for half in range(2):
                        ks = slice(half * 512, (half + 1) * 512)
                        nc.tensor.matmul(sp[0][:, ks], qT_sb[hp][r0, qsl],
                                         kT_sb[hp][r0, ks])
                        nc.tensor.matmul(sp[1][:, ks], qT_sb[hp][r1, qsl],
                                         kT_sb[hp][r1, ks])
                    # d_all column layout: hp*8 + qc*2 + head-parity (so the
                    # two heads' sums sit adjacent for one batched reciprocal,
                    # and the per-hp transpose slice stays contiguous)
                    col0 = hp * 2 * QC + qc * 2
                    pts_ = []
                    for i, h in ((0, h0), (1, h1)):
                        if use_mask:
                            nc.vector.tensor_tensor(out=sp[i], in0=sp[i],
                                                    in1=mrep_sb, op=ALU.add)
                        pt = probsp.tile([P, S], f32, tag="probs", name="probs")
                        nc.scalar.activation(
                            out=pt, in_=sp[i], func=AF.Exp, scale=SCALE,
                            accum_out=d_all[:, col0 + i:col0 + i + 1],
                        )
                        pts_.append(pt)
                    nc.vector.reciprocal(out=inv_all[:, col0:col0 + 2],
                                         in_=d_all[:, col0:col0 + 2])
                    for i, h in ((0, h0), (1, h1)):
                        nc.vector.tensor_scalar_mul(
                            out=pts_[i], in0=pts_[i],
                            scalar1=inv_all[:, col0 + i:col0 + i + 1])
                        nc.sync.dma_start(out=probs_o[h, qsl, :], in_=pts_[i])

                # 1/d transposed to rows -> DRAM (broadcast read in st_ctx)
                dts = psC.tile([P, QL], f32, tag="cx", name="dts")
                nc.tensor.transpose(dts[0:2 * QC, 0:P],
                                    d_all[:, hp * 2 * QC:(hp + 1) * 2 * QC],
                                    ident)
                invdt = statp.tile([2 * QC, P], f32, tag="invdt")
                nc.vector.reciprocal(out=invdt, in_=dts[0:2 * QC, 0:P])
                nc.sync.dma_start(
                    out=invd_dram[hp * 2 * QC:(hp + 1) * 2 * QC, :], in_=invdt)

            # ---- stage A: projections, with the probs (S-side) work of
            # head pair c-1 interleaved so ScalarE fills during stage A ----
            # kT_sb[c]: K^T cols chunk c -> [d_col 128, s 1024]
            # qT_sb[c]: Q^T              -> [d_col 128, q 512]
            # v_sb[c]:  V rows chunk c   -> [s 128, d_col 1024]
            kT_sb, qT_sb, v_sb = [], [], []
            with tc.tile_pool(name="stageA", bufs=1) as sap:
                # first projection group (kps c=0) needs all of wk+hsT, so
                # those loads go first, interleaved to spread across queues
                wk_sb, wk_th = load_chunks(sap, "wk", wk, D, defer=True)
                hsT_sb, hsT_th = load_chunks(sap, "hsT", hsT, S, defer=True)
                wq_sb, wq_th = load_chunks(sap, "wq", wq, D, defer=True)
                hsqT_sb, hsqT_th = load_chunks(sap, "hsqT", hsqT, QL,
                                               defer=True)
                wv_sb, wv_th = load_chunks(sap, "wv", wv, D, defer=True)
                for dd in range(DC):
                    hsT_th[dd]()
                    wk_th[dd]()
                for dd in range(DC):
                    wq_th[dd]()
                    hsqT_th[dd]()
                for dd in range(DC):
                    wv_th[dd]()


                for c in range(DC):
                    kt = singles.tile([P, S], bf16, tag=f"kT{c}", name=f"kT{c}")
                    for half in range(2):
                        sl = slice(half * 512, (half + 1) * 512)
                        kps = psP.tile([P, 512], f32, tag="proj", name="kps")
                        for dd in range(DC):
                            nc.tensor.matmul(
                                kps,
                                wk_sb[dd][:, c * P:(c + 1) * P],
                                hsT_sb[dd][:, sl],
                                start=(dd == 0), stop=(dd == DC - 1),
                            )
                        nc.vector.tensor_scalar_add(out=kt[:, sl], in0=kps,
                                                    scalar1=bkr_sb[:, c:c + 1])
                    kT_sb.append(kt)

                    qps = psP.tile([P, QL], f32, tag="proj", name="qps")
                    for dd in range(DC):
                        nc.tensor.matmul(
                            qps, wq_sb[dd][:, c * P:(c + 1) * P], hsqT_sb[dd],
                            start=(dd == 0), stop=(dd == DC - 1),
                        )
                    qt = singles.tile([P, QL], bf16, tag=f"qT{c}", name=f"qT{c}")
                    nc.vector.tensor_scalar_add(out=qt, in0=qps,
                                                scalar1=bqr_sb[:, c:c + 1])
                    qT_sb.append(qt)

                    # attention S-side for this very chunk: keeps ScalarE fed
                    # throughout stage A
                    s_side(c)

                # V projections last: V is only needed by the phase-2 ctx
                # matmuls. The wait-until timestamp stops the scheduler from
                # pulling this dense block early; it lands where ScalarE is
                # still catching up on the s_side exp backlog, keeping PE
                # busy enough that the HAM clock gate stays at full rate.
                for c in range(DC):
                    vt = singles.tile([P, D], bf16, tag=f"v{c}", name=f"v{c}")
                    with tc.tile_wait_until(0.075):
                        for half in range(2):
                            sl = slice(half * 512, (half + 1) * 512)
                            vps = psP.tile([P, 512], f32, tag="proj",
                                           name="vps")
                            for dd in range(DC):
                                nc.tensor.matmul(
                                    vps,
                                    hsT_sb[dd][:, c * P:(c + 1) * P],
                                    wv_sb[dd][:, sl],
                                    start=(dd == 0), stop=(dd == DC - 1),
                                )
                            if use_bv:
                                nc.vector.tensor_tensor(out=vt[:, sl],
                                                        in0=vps,
                                                        in1=bv_sb[:, sl],
                                                        op=ALU.add)
                            else:
                                nc.vector.tensor_copy(out=vt[:, sl], in_=vps)
                    v_sb.append(vt)

            # stage-C tensors reuse the space stage A released
            latep = ctx.enter_context(tc.tile_pool(name="latep", bufs=1))
            xp = ctx.enter_context(tc.tile_pool(name="xp", bufs=2))
            wo_sb = load_chunks(latep, "wo", wo, D)
            # residual (+bo) rows and SBUF accumulators for the output
            # projection, which runs incrementally (one dd-pair batch after
            # every second head pair) on the otherwise-idle psP slots, so
            # only the last quarter of it remains in the stage-C tail
            res_sb, oacc = [], []
            for qc in range(QC):
                rt = latep.tile([P, D], f32, tag=f"res{qc}", name=f"res{qc}")
                nc.sync.dma_start(out=rt, in_=res[qc * P:(qc + 1) * P, :])
                res_sb.append(rt)
                oacc.append(latep.tile([P, D], f32, tag=f"oacc{qc}",
                                       name=f"oacc{qc}"))

            def st_ctx(hp):
                """S^T -> exp -> P^T; ctx^T = V^T @ P^T, scaled by 1/d."""
                invd_rep = invdp.tile([P, QC, P], f32, tag="invd_rep",
                                      name="invd_rep")
                for par in range(2):
                    src = bass.AP(
                        tensor=invd_dram.tensor,
                        offset=invd_dram.offset + (hp * 2 * QC + par) * P,
                        ap=[[0, DH], [2 * P, QC], [1, P]],
                    )
                    nc.gpsimd.dma_start(out=invd_rep[par * DH:(par + 1) * DH],
                                        in_=src)
                h0, h1 = 2 * hp, 2 * hp + 1
                r0, r1 = slice(0, DH), slice(DH, 2 * DH)
                cxs = psC.tile([P, QL], f32, tag="cx", name="cxs")
                pts = {}

                def ctx_mms(kc):
                    # PE is in-order: emitting ctx(kc) right after exp(kc)
                    # would make PE block on ScalarE each step. Lag by 2 so
                    # the exp is long done when PE reaches the ctx matmuls.
                    pt2 = pts.pop(kc)
                    nc.tensor.matmul(cxs[0:DH, :],
                                     v_sb[kc][:, h0 * DH:(h0 + 1) * DH],
                                     pt2[:, 0:QL],
                                     start=(kc == 0), stop=(kc == DC - 1))
                    nc.tensor.matmul(cxs[DH:2 * DH, :],
                                     v_sb[kc][:, h1 * DH:(h1 + 1) * DH],
                                     pt2[:, QL:2 * QL],
                                     start=(kc == 0), stop=(kc == DC - 1))

                for kc in range(DC):
                    ksl = slice(kc * P, (kc + 1) * P)
                    st = psS.tile([P, 2 * QL], f32, tag="sp", name="st")
                    nc.tensor.matmul(st[:, 0:QL], kT_sb[hp][r0, ksl],
                                     qT_sb[hp][r0, :])
                    nc.tensor.matmul(st[:, QL:2 * QL], kT_sb[hp][r1, ksl],
                                     qT_sb[hp][r1, :])
                    if use_mask:
                        nc.vector.tensor_scalar_add(
                            out=st, in0=st, scalar1=mcol_sb[:, kc:kc + 1])
                    pt2 = ptp.tile([P, 2 * QL], bf16, tag=f"pt{kc}",
                                   name=f"pt{kc}")
                    nc.scalar.activation(out=pt2, in_=st, func=AF.Exp,
                                         scale=SCALE)
                    pts[kc] = pt2
                    if kc >= 2:
                        ctx_mms(kc - 2)
                ctx_mms(DC - 2)
                ctx_mms(DC - 1)
                cxt = singles.tile([P, QL], bf16, tag=f"ctxT{hp}",
                                   name=f"ctxT{hp}")
                nc.vector.tensor_tensor(
                    out=cxt, in0=cxs,
                    in1=invd_rep.rearrange("p a b -> p (a b)"), op=ALU.mult)
                return cxt

            ctxT_sb = []
            for hp in range(HP):
                ctxT_sb.append(st_ctx(hp))
                if hp % 2 == 1:
                    batch = hp // 2
                    for qc in range(QC):
                        qsl_ = slice(qc * P, (qc + 1) * P)
                        for half in range(2):
                            sl = slice(half * 512, (half + 1) * 512)
                            pps = psP.tile([P, 512], f32, tag="proj",
                                           name="pps")
                            for j, dd in enumerate((2 * batch, 2 * batch + 1)):
                                nc.tensor.matmul(
                                    pps, ctxT_sb[dd][:, qsl_],
                                    wo_sb[dd][:, sl],
                                    start=(j == 0), stop=(j == 1))
                            if batch == 0:
                                nc.vector.tensor_tensor(
                                    out=oacc[qc][:, sl], in0=pps,
                                    in1=res_sb[qc][:, sl], op=ALU.add)
                            else:
                                nc.vector.tensor_tensor(
                                    out=oacc[qc][:, sl],
                                    in0=oacc[qc][:, sl], in1=pps,
                                    op=ALU.add)

            # ---- stage C: LayerNorm epilogue (out-proj already folded) ----
            for qc in range(QC):
                qsl = slice(qc * P, (qc + 1) * P)
                x = oacc[qc]
                stats = statp.tile([P, 2, 6], f32, tag="stats", name="stats")
                for i in range(2):
                    nc.vector.bn_stats(out=stats[:, i, :],
                                       in_=x[:, i * 512:(i + 1) * 512])
                mv = statp.tile([P, 2], f32, tag="mv", name="mv")
                nc.vector.bn_aggr(out=mv, in_=stats)
                std = statp.tile([P, 1], f32, tag="std", name="std")
                nc.scalar.activation(out=std, in_=mv[:, 1:2], func=AF.Sqrt,
                                     bias=eps_t, scale=1.0)
                rstd = statp.tile([P, 1], f32, tag="rstd", name="rstd")
                nc.vector.reciprocal(out=rstd, in_=std)
                ot = xp.tile([P, D], f32, tag="ot", name="ot")
                nc.vector.tensor_scalar(
                    out=ot, in0=x, scalar1=mv[:, 0:1], scalar2=rstd,
                    op0=ALU.subtract, op1=ALU.mult,
                )
                if use_gamma:
                    nc.vector.tensor_tensor(out=ot, in0=ot, in1=g_sb,
                                            op=ALU.mult)
                if use_beta:
                    nc.vector.tensor_tensor(out=ot, in0=ot, in1=be_sb,
                                            op=ALU.add)
                nc.sync.dma_start(out=out_o[qsl, :], in_=ot)

    _split_wide_waits(nc, mybir)
    return nc


def _get_nc(flags):
    if flags not in _BUILD_CACHE:
        _BUILD_CACHE[flags] = _build(*flags)
    return _BUILD_CACHE[flags]


def _prep(hidden_states, attention_mask, wq, bq, wk, bk, wv, bv, wo, bo,
          ln_gamma, ln_beta):
    """Returns (flags, in_maps)."""
    bf16 = ml_dtypes.bfloat16
    hs = np.asarray(hidden_states, dtype=np.float32)
    mask = np.asarray(attention_mask)
    wq_ = np.asarray(wq, dtype=np.float32).astype(bf16)
    wk_ = np.asarray(wk, dtype=np.float32).astype(bf16)
    wv_ = np.asarray(wv, dtype=np.float32).astype(bf16)
    wo_ = np.asarray(wo, dtype=np.float32).astype(bf16)
    bq_ = np.asarray(bq, dtype=np.float32)
    bk_ = np.asarray(bk, dtype=np.float32)
    bv_ = np.asarray(bv, dtype=np.float32)
    bo_ = np.asarray(bo, dtype=np.float32)
    g_ = np.asarray(ln_gamma, dtype=np.float32)
    be_ = np.asarray(ln_beta, dtype=np.float32)

    use_bv = bool(np.any(bv_ != 0.0))
    use_mask = bool(np.any(mask != 1))
    use_gamma = bool(np.any(g_ != 1.0))
    use_beta = bool(np.any(be_ != 0.0))
    flags = (use_bv, use_mask, use_gamma, use_beta)

    bqr = np.ascontiguousarray(bq_.reshape(DC, P).T)
    bkr = np.ascontiguousarray(bk_.reshape(DC, P).T)

    in_maps = []
    hsT_b = {}
    for b in range(B):
        hsT_b[b] = np.ascontiguousarray(hs[b].T).astype(bf16)
    for c in range(NCORES):
        b, qh = c // 2, c % 2
        q0 = qh * QL
        m = {
            "hsT": hsT_b[b],
            "hsqT": np.ascontiguousarray(hsT_b[b][:, q0:q0 + QL]),
            "res": hs[b, q0:q0 + QL] + bo_,
            "wq": wq_, "wk": wk_, "wv": wv_, "wo": wo_,
            "bqr": bqr, "bkr": bkr,
        }
        if use_bv:
            m["bvrep"] = np.ascontiguousarray(
                np.broadcast_to(bv_, (P, D)).astype(np.float32))
        if use_mask:
            mb = (1.0 - mask[b].astype(np.float32)) * (-10000.0 * 32.0)
            m["mrep"] = np.ascontiguousarray(np.broadcast_to(mb, (P, S)))
            m["mcol"] = np.ascontiguousarray(mb.reshape(DC, P).T)
        if use_gamma:
            m["grep"] = np.ascontiguousarray(np.broadcast_to(g_, (P, D)))
        if use_beta:
            m["brep"] = np.ascontiguousarray(np.broadcast_to(be_, (P, D)))
        in_maps.append(m)
    return flags, in_maps


def kernel(**inputs):
    from concourse.bass_utils import run_bass_kernel_spmd

    flags, in_maps = _prep(**inputs)
    nc = _get_nc(flags)
    r = run_bass_kernel_spmd(nc, in_maps, core_ids=list(range(NCORES)))

    out = np.empty((B, S, D), np.float32)
    probs = np.empty((B, H, S, S), np.float32)
    for c in range(NCORES):
        b, qh = c // 2, c % 2
        q0 = qh * QL
        out[b, q0:q0 + QL] = r.results[c]["out"]
        probs[b, :, q0:q0 + QL, :] = r.results[c]["probs"]
    return out, probs


# revision 53
# speedup vs baseline: 1.2538x; 1.2538x over previous
"""ALBERT attention layer (B=4, S=1024, D=1024, H=16) on 8 Trainium2 NeuronCores.

Sharding: core c = (batch b = c//2, query-half qh = c%2). Each core computes
the attention output rows q0 = qh*512 .. q0+512 of its batch: it projects
K/V for the full batch (duplicated across the 2 cores sharing a batch --
cheaper than a collective), Q for its own rows only, and produces its slice
of both outputs (out[b, q0:q0+512, :] and probs[b, :, q0:q0+512, :]).

Layout strategy per core:
  - hidden_states arrives host-pre-transposed (hsT, [D, S], bf16) so that
    K^T/Q^T ([d_col, s]) and V ([s, d_col]) all come straight out of the
    tensor engine with no on-device transposes.
  - scores are computed twice: once as S[q, k] (softmax over the free dim,
    exp with fused row-sum accum_out -> f32 probs output), and once as
    S^T[k, q] (row-tiled K=64 matmul pairs) whose exp gives the bf16
    operand P^T for the context matmul, which directly yields ctx^T --
    exactly the lhsT the output projection needs.
  - softmax 1/sum for the ctx path is transposed once per head-pair via the
    tensor engine ([128, 8] -> [8, 128]) and broadcast across partitions
    with a tiny DRAM round-trip.
  - head pairs are software-pipelined: the S-side (probs) work of pair hp
    is emitted alongside the S^T/ctx work of pair hp-1 so ScalarE (the
    long pole: 128 exp passes) always has runway and PE never idles long
    enough for the HAM clock gate to drop it to half rate.
"""

import numpy as np
import ml_dtypes

B, S, D, H = 4, 1024, 1024, 16
DH = D // H           # 64
P = 128
QL = S // 2           # 512 query rows per core
NCORES = 2 * B        # 8
DC = D // P           # 8 column chunks
QC = QL // P          # 4 query chunks per core
HP = H // 2           # 8 head pairs
SCALE = 1.0 / 32.0    # ALBERT: 1/sqrt(hidden_size) = 1/sqrt(1024)
EPS = 1e-12

_BUILD_CACHE = {}


def _split_wide_waits(nc, mybir, max_waits=1):
    """This walrus build rejects instructions carrying more than one sync
    wait (TPB_CTRL-class ops at least; the Tile kernel-tail drain aggregates
    one per busy proc). Move excess waits onto preceding same-engine NOPs --
    semantically identical since the engine executes its stream in order."""
    n_split = 0
    for f in nc.m.functions:
        for bb in f.blocks:
            insts = list(bb.instructions)
            out = []
            changed = False
            for inst in insts:
                si = getattr(inst, "sync_info", None)
                waits = list(si.on_wait) if (si is not None and si.on_wait) else []
                if len(waits) > max_waits:
                    n_split += 1
                    changed = True
                    k = 0
                    while len(waits) - k > max_waits:
                        chunk = waits[k:k + max_waits]
                        k += max_waits
                        nop = mybir.InstNoOp(
                            name=nc.get_next_instruction_name(),
                            sync_info=mybir.SyncInfo(on_wait=chunk, on_update=[]),
                            bass_nofuse=True,
                            engine=inst.engine,
                        )
                        nc.register_instruction(nop)
                        out.append(nop)
                    si.on_wait = waits[k:]
                out.append(inst)
            if changed:
                bb.instructions = out
    return n_split


def _build(use_bv, use_mask, use_gamma, use_beta):
    import concourse.bass as bass
    import concourse.mybir as mybir
    import concourse.tile as tile
    from concourse.masks import make_identity
    from contextlib import ExitStack

    f32 = mybir.dt.float32
    bf16 = mybir.dt.bfloat16
    AF = mybir.ActivationFunctionType
    ALU = mybir.AluOpType

    nc = bass.Bass("TRN2", target_bir_lowering=False, debug=False,
                   num_devices=NCORES)

    # ---- I/O ----
    hsT = nc.dram_tensor("hsT", [D, S], bf16, kind="ExternalInput")
    hsqT = nc.dram_tensor("hsqT", [D, QL], bf16, kind="ExternalInput")
    res = nc.dram_tensor("res", [QL, D], f32, kind="ExternalInput")
    wq = nc.dram_tensor("wq", [D, D], bf16, kind="ExternalInput")
    wk = nc.dram_tensor("wk", [D, D], bf16, kind="ExternalInput")
    wv = nc.dram_tensor("wv", [D, D], bf16, kind="ExternalInput")
    wo = nc.dram_tensor("wo", [D, D], bf16, kind="ExternalInput")
    bqr = nc.dram_tensor("bqr", [P, DC], f32, kind="ExternalInput")
    bkr = nc.dram_tensor("bkr", [P, DC], f32, kind="ExternalInput")
    if use_bv:
        bvrep = nc.dram_tensor("bvrep", [P, D], f32, kind="ExternalInput")
    if use_mask:
        # 32 * (-10000) * (1 - mask[k]): row-replicated for the S[q,k] psum
        # add, and per-partition columns for the S^T[k,q] psum add.
        mrep = nc.dram_tensor("mrep", [P, S], f32, kind="ExternalInput")
        mcol = nc.dram_tensor("mcol", [P, DC], f32, kind="ExternalInput")
    if use_gamma:
        grep = nc.dram_tensor("grep", [P, D], f32, kind="ExternalInput")
    if use_beta:
        brep = nc.dram_tensor("brep", [P, D], f32, kind="ExternalInput")
    probs_o = nc.dram_tensor("probs", [H, QL, S], f32, kind="ExternalOutput")
    out_o = nc.dram_tensor("out", [QL, D], f32, kind="ExternalOutput")

    with tile.TileContext(nc) as tc:
        with ExitStack() as ctx:
            singles = ctx.enter_context(tc.tile_pool(name="singles", bufs=1))
            probsp = ctx.enter_context(tc.tile_pool(name="probsp", bufs=6))
            # ptp/invdp live OUTSIDE the stageA region: phase-2 tiles must
            # not inherit released-zone deps on stage A's last weight reads
            # (that would serialize the phase transition and cool the PE).
            # Stage-C-only tensors (wo/res/x) go into the released region
            # instead -- their late deps are harmless.
            ptp = ctx.enter_context(tc.tile_pool(name="ptp", bufs=2))
            invdp = ctx.enter_context(tc.tile_pool(name="invdp", bufs=2))
            statp = ctx.enter_context(tc.tile_pool(name="statp", bufs=4))
            # PSUM budget (16KB/partition): proj 3x1 + sp 2x2 + cx 1x1 banks
            psP = ctx.enter_context(tc.tile_pool(name="psP", bufs=3, space="PSUM"))
            psS = ctx.enter_context(tc.tile_pool(name="psS", bufs=2, space="PSUM"))
            psC = ctx.enter_context(tc.tile_pool(name="psC", bufs=1, space="PSUM"))
            dramp = ctx.enter_context(tc.tile_pool(name="dramp", bufs=1, space="DRAM"))

            def load_chunks(pool, name, src, width, defer=False):
                """Allocate chunk tiles; if defer, return the DMA thunks so
                the caller can sequence loads in dependency-priority order."""
                tiles, thunks = [], []
                for dd in range(DC):
                    t = pool.tile([P, width], bf16, tag=f"{name}{dd}",
                                  name=f"{name}{dd}")
                    lo, hi = dd * P, (dd + 1) * P
                    thunks.append(
                        lambda t=t, lo=lo, hi=hi: nc.sync.dma_start(
                            out=t, in_=src[lo:hi, :]))
                    tiles.append(t)
                if not defer:
                    for th in thunks:
                        th()
                return (tiles, thunks) if defer else tiles

            bqr_sb = singles.tile([P, DC], f32, tag="bqr")
            nc.sync.dma_start(out=bqr_sb, in_=bqr[:, :])
            bkr_sb = singles.tile([P, DC], f32, tag="bkr")
            nc.sync.dma_start(out=bkr_sb, in_=bkr[:, :])
            if use_bv:
                bv_sb = singles.tile([P, D], f32, tag="bvrep")
                nc.sync.dma_start(out=bv_sb, in_=bvrep[:, :])
            if use_mask:
                mrep_sb = singles.tile([P, S], f32, tag="mrep")
                nc.sync.dma_start(out=mrep_sb, in_=mrep[:, :])
                mcol_sb = singles.tile([P, DC], f32, tag="mcol")
                nc.sync.dma_start(out=mcol_sb, in_=mcol[:, :])
            if use_gamma:
                g_sb = singles.tile([P, D], f32, tag="grep")
                nc.sync.dma_start(out=g_sb, in_=grep[:, :])
            if use_beta:
                be_sb = singles.tile([P, D], f32, tag="brep")
                nc.sync.dma_start(out=be_sb, in_=brep[:, :])

            ident = singles.tile([P, P], f32, tag="ident")
            make_identity(nc, ident)
            eps_t = singles.tile([P, 1], f32, tag="eps")
            nc.vector.memset(eps_t, EPS)

            # per-(head, q-chunk) exp row sums: column h*QC + qc
            d_all = singles.tile([P, H * QC], f32, tag="d_all")
            inv_all = singles.tile([P, H * QC], f32, tag="inv_all")
            invd_dram = dramp.tile([H * QC, P], f32)

            def s_side(hp):
                """S[q,k] -> exp/softmax -> probs rows; leaves 1/d (row
                layout) in invd_dram for st_ctx to broadcast later."""
                h0, h1 = 2 * hp, 2 * hp + 1
                r0, r1 = slice(0, DH), slice(DH, 2 * DH)
                for qc in range(QC):
                    qsl = slice(qc * P, (qc + 1) * P)
                    sp = [psS.tile([P, S], f32, tag="sp", name=f"sp{i}")
                          for i in range(2)]
                    for half in range(2):
                        ks = slice(half * 512, (half + 1) * 512)
                        nc.tensor.matmul(sp[0][:, ks], qT_sb[hp][r0, qsl],
                                         kT_sb[hp][r0, ks])
                        nc.tensor.matmul(sp[1][:, ks], qT_sb[hp][r1, qsl],
                                         kT_sb[hp][r1, ks])
                    # d_all column layout: hp*8 + qc*2 + head-parity (so the
                    # two heads' sums sit adjacent for one batched reciprocal,
                    # and the per-hp transpose slice stays contiguous)
                    col0 = hp * 2 * QC + qc * 2
                    pts_ = []
                    for i, h in ((0, h0), (1, h1)):
                        if use_mask:
                            nc.vector.tensor_tensor(out=sp[i], in0=sp[i],
                                                    in1=mrep_sb, op=ALU.add)
                        pt = probsp.tile([P, S], f32, tag="probs", name="probs")
                        nc.scalar.activation(
                            out=pt, in_=sp[i], func=AF.Exp, scale=SCALE,
                            accum_out=d_all[:, col0 + i:col0 + i + 1],
                        )
                        pts_.append(pt)
                    nc.vector.reciprocal(out=inv_all[:, col0:col0 + 2],
                                         in_=d_all[:, col0:col0 + 2])
                    for i, h in ((0, h0), (1, h1)):
                        nc.vector.tensor_scalar_mul(
                            out=pts_[i], in0=pts_[i],
                            scalar1=inv_all[:, col0 + i:col0 + i + 1])
                        nc.sync.dma_start(out=probs_o[h, qsl, :], in_=pts_[i])

                # 1/d transposed to rows -> DRAM (broadcast read in st_ctx)
                dts = psC.tile([P, QL], f32, tag="cx", name="dts")
                nc.tensor.transpose(dts[0:2 * QC, 0:P],
                                    d_all[:, hp * 2 * QC:(hp + 1) * 2 * QC],
                                    ident)
                invdt = statp.tile([2 * QC, P], f32, tag="invdt")
                nc.vector.reciprocal(out=invdt, in_=dts[0:2 * QC, 0:P])
                nc.sync.dma_start(
                    out=invd_dram[hp * 2 * QC:(hp + 1) * 2 * QC, :], in_=invdt)

            # ---- stage A: projections, with the probs (S-side) work of
            # head pair c-1 interleaved so ScalarE fills during stage A ----
            # kT_sb[c]: K^T cols chunk c -> [d_col 128, s 1024]
            # qT_sb[c]: Q^T              -> [d_col 128, q 512]
            # v_sb[c]:  V rows chunk c   -> [s 128, d_col 1024]
            kT_sb, qT_sb, v_sb = [], [], []
            with tc.tile_pool(name="stageA", bufs=1) as sap:
                # first projection group (kps c=0) needs all of wk+hsT, so
                # those loads go first, interleaved to spread across queues
                wk_sb, wk_th = load_chunks(sap, "wk", wk, D, defer=True)
                hsT_sb, hsT_th = load_chunks(sap, "hsT", hsT, S, defer=True)
                wq_sb, wq_th = load_chunks(sap, "wq", wq, D, defer=True)
                hsqT_sb, hsqT_th = load_chunks(sap, "hsqT", hsqT, QL,
                                               defer=True)
                wv_sb, wv_th = load_chunks(sap, "wv", wv, D, defer=True)
                for dd in range(DC):
                    hsT_th[dd]()
                    wk_th[dd]()
                for dd in range(DC):
                    wq_th[dd]()
                    hsqT_th[dd]()
                for dd in range(DC):
                    wv_th[dd]()


                for c in range(DC):
                    kt = singles.tile([P, S], bf16, tag=f"kT{c}", name=f"kT{c}")
                    for half in range(2):
                        sl = slice(half * 512, (half + 1) * 512)
                        kps = psP.tile([P, 512], f32, tag="proj", name="kps")
                        for dd in range(DC):
                            nc.tensor.matmul(
                                kps,
                                wk_sb[dd][:, c * P:(c + 1) * P],
                                hsT_sb[dd][:, sl],
                                start=(dd == 0), stop=(dd == DC - 1),
                            )
                        nc.vector.tensor_scalar_add(out=kt[:, sl], in0=kps,
                                                    scalar1=bkr_sb[:, c:c + 1])
                    kT_sb.append(kt)

                    qps = psP.tile([P, QL], f32, tag="proj", name="qps")
                    for dd in range(DC):
                        nc.tensor.matmul(
                            qps, wq_sb[dd][:, c * P:(c + 1) * P], hsqT_sb[dd],
                            start=(dd == 0), stop=(dd == DC - 1),
                        )
                    qt = singles.tile([P, QL], bf16, tag=f"qT{c}", name=f"qT{c}")
                    nc.vector.tensor_scalar_add(out=qt, in0=qps,
                                                scalar1=bqr_sb[:, c:c + 1])
                    qT_sb.append(qt)

                    # attention S-side for this very chunk: keeps ScalarE fed
                    # throughout stage A
                    s_side(c)

                # V projections last: V is only needed by the phase-2 ctx
                # matmuls, and this dense block keeps PE hot (HAM at full
                # clock) exactly while ScalarE catches up on the s_side
                # exp backlog.
                for c in range(DC):
                    vt = singles.tile([P, D], bf16, tag=f"v{c}", name=f"v{c}")
                    for half in range(2):
                        sl = slice(half * 512, (half + 1) * 512)
                        vps = psP.tile([P, 512], f32, tag="proj", name="vps")
                        for dd in range(DC):
                            nc.tensor.matmul(
                                vps,
                                hsT_sb[dd][:, c * P:(c + 1) * P],
                                wv_sb[dd][:, sl],
                                start=(dd == 0), stop=(dd == DC - 1),
                            )
                        if use_bv:
                            nc.vector.tensor_tensor(out=vt[:, sl], in0=vps,
                                                    in1=bv_sb[:, sl],
                                                    op=ALU.add)
                        else:
                            nc.vector.tensor_copy(out=vt[:, sl], in_=vps)
                    v_sb.append(vt)

            # stage-C tensors reuse the space stage A released
            latep = ctx.enter_context(tc.tile_pool(name="latep", bufs=1))
            xp = ctx.enter_context(tc.tile_pool(name="xp", bufs=2))
            wo_sb = load_chunks(latep, "wo", wo, D)
            # residual (+bo) rows and SBUF accumulators for the output
            # projection, which runs incrementally (one dd-pair batch after
            # every second head pair) on the otherwise-idle psP slots, so
            # only the last quarter of it remains in the stage-C tail
            res_sb, oacc = [], []
            for qc in range(QC):
                rt = latep.tile([P, D], f32, tag=f"res{qc}", name=f"res{qc}")
                nc.sync.dma_start(out=rt, in_=res[qc * P:(qc + 1) * P, :])
                res_sb.append(rt)
                oacc.append(latep.tile([P, D], f32, tag=f"oacc{qc}",
                                       name=f"oacc{qc}"))

            def st_ctx(hp):
                """S^T -> exp -> P^T; ctx^T = V^T @ P^T, scaled by 1/d."""
                invd_rep = invdp.tile([P, QC, P], f32, tag="invd_rep",
                                      name="invd_rep")
                for par in range(2):
                    src = bass.AP(
                        tensor=invd_dram.tensor,
                        offset=invd_dram.offset + (hp * 2 * QC + par) * P,
                        ap=[[0, DH], [2 * P, QC], [1, P]],
                    )
                    nc.gpsimd.dma_start(out=invd_rep[par * DH:(par + 1) * DH],
                                        in_=src)
                h0, h1 = 2 * hp, 2 * hp + 1
                r0, r1 = slice(0, DH), slice(DH, 2 * DH)
                cxs = psC.tile([P, QL], f32, tag="cx", name="cxs")
                pts = {}

                def ctx_mms(kc):
                    # PE is in-order: emitting ctx(kc) right after exp(kc)
                    # would make PE block on ScalarE each step. Lag by 2 so
                    # the exp is long done when PE reaches the ctx matmuls.
                    pt2 = pts.pop(kc)
                    nc.tensor.matmul(cxs[0:DH, :],
                                     v_sb[kc][:, h0 * DH:(h0 + 1) * DH],
                                     pt2[:, 0:QL],
                                     start=(kc == 0), stop=(kc == DC - 1))
                    nc.tensor.matmul(cxs[DH:2 * DH, :],
                                     v_sb[kc][:, h1 * DH:(h1 + 1) * DH],
                                     pt2[:, QL:2 * QL],
                                     start=(kc == 0), stop=(kc == DC - 1))

                for kc in range(DC):
                    ksl = slice(kc * P, (kc + 1) * P)
                    st = psS.tile([P, 2 * QL], f32, tag="sp", name="st")
                    nc.tensor.matmul(st[:, 0:QL], kT_sb[hp][r0, ksl],
                                     qT_sb[hp][r0, :])
                    nc.tensor.matmul(st[:, QL:2 * QL], kT_sb[hp][r1, ksl],
                                     qT_sb[hp][r1, :])
                    if use_mask:
                        nc.vector.tensor_scalar_add(
                            out=st, in0=st, scalar1=mcol_sb[:, kc:kc + 1])
                    pt2 = ptp.tile([P, 2 * QL], bf16, tag=f"pt{kc}",
                                   name=f"pt{kc}")
                    nc.scalar.activation(out=pt2, in_=st, func=AF.Exp,
                                         scale=SCALE)
                    pts[kc] = pt2
                    if kc >= 2:
                        ctx_mms(kc - 2)
                ctx_mms(DC - 2)
                ctx_mms(DC - 1)
                cxt = singles.tile([P, QL], bf16, tag=f"ctxT{hp}",
                                   name=f"ctxT{hp}")
                nc.vector.tensor_tensor(
                    out=cxt, in0=cxs,
                    in1=invd_rep.rearrange("p a b -> p (a b)"), op=ALU.mult)
                return cxt

            ctxT_sb = []
            for hp in range(HP):
                ctxT_sb.append(st_ctx(hp))
                if hp % 2 == 1:
                    batch = hp // 2
                    for qc in range(QC):
                        qsl_ = slice(qc * P, (qc + 1) * P)
                        for half in range(2):
                            sl = slice(half * 512, (half + 1) * 512)
                            pps = psP.tile([P, 512], f32, tag="proj",
                                           name="pps")
                            for j, dd in enumerate((2 * batch, 2 * batch + 1)):
                                nc.tensor.matmul(
                                    pps, ctxT_sb[dd][:, qsl_],
                                    wo_sb[dd][:, sl],
                                    start=(j == 0), stop=(j == 1))
                            if batch == 0:
                                nc.vector.tensor_tensor(
                                    out=oacc[qc][:, sl], in0=pps,
                                    in1=res_sb[qc][:, sl], op=ALU.add)
                            else:
                                nc.vector.tensor_tensor(
                                    out=oacc[qc][:, sl],
                                    in0=oacc[qc][:, sl], in1=pps,
                                    op=ALU.add)

            # ---- stage C: LayerNorm epilogue (out-proj already folded) ----
            for qc in range(QC):
                qsl = slice(qc * P, (qc + 1) * P)
                x = oacc[qc]
                stats = statp.tile([P, 2, 6], f32, tag="stats", name="stats")
                for i in range(2):
                    nc.vector.bn_stats(out=stats[:, i, :],
                                       in_=x[:, i * 512:(i + 1) * 512])
                mv = statp.tile([P, 2], f32, tag="mv", name="mv")
                nc.vector.bn_aggr(out=mv, in_=stats)
                std = statp.tile([P, 1], f32, tag="std", name="std")
                nc.scalar.activation(out=std, in_=mv[:, 1:2], func=AF.Sqrt,
                                     bias=eps_t, scale=1.0)
                rstd = statp.tile([P, 1], f32, tag="rstd", name="rstd")
                nc.vector.reciprocal(out=rstd, in_=std)
                ot = xp.tile([P, D], f32, tag="ot", name="ot")
                nc.vector.tensor_scalar(
                    out=ot, in0=x, scalar1=mv[:, 0:1], scalar2=rstd,
                    op0=ALU.subtract, op1=ALU.mult,
                )
                if use_gamma:
                    nc.vector.tensor_tensor(out=ot, in0=ot, in1=g_sb,
                                            op=ALU.mult)
                if use_beta:
                    nc.vector.tensor_tensor(out=ot, in0=ot, in1=be_sb,
                                            op=ALU.add)
                nc.sync.dma_start(out=out_o[qsl, :], in_=ot)

    _split_wide_waits(nc, mybir)
    return nc


def _get_nc(flags):
    if flags not in _BUILD_CACHE:
        _BUILD_CACHE[flags] = _build(*flags)
    return _BUILD_CACHE[flags]


def _prep(hidden_states, attention_mask, wq, bq, wk, bk, wv, bv, wo, bo,
          ln_gamma, ln_beta):
    """Returns (flags, in_maps)."""
    bf16 = ml_dtypes.bfloat16
    hs = np.asarray(hidden_states, dtype=np.float32)
    mask = np.asarray(attention_mask)
    wq_ = np.asarray(wq, dtype=np.float32).astype(bf16)
    wk_ = np.asarray(wk, dtype=np.float32).astype(bf16)
    wv_ = np.asarray(wv, dtype=np.float32).astype(bf16)
    wo_ = np.asarray(wo, dtype=np.float32).astype(bf16)
    bq_ = np.asarray(bq, dtype=np.float32)
    bk_ = np.asarray(bk, dtype=np.float32)
    bv_ = np.asarray(bv, dtype=np.float32)
    bo_ = np.asarray(bo, dtype=np.float32)
    g_ = np.asarray(ln_gamma, dtype=np.float32)
    be_ = np.asarray(ln_beta, dtype=np.float32)

    use_bv = bool(np.any(bv_ != 0.0))
    use_mask = bool(np.any(mask != 1))
    use_gamma = bool(np.any(g_ != 1.0))
    use_beta = bool(np.any(be_ != 0.0))
    flags = (use_bv, use_mask, use_gamma, use_beta)

    bqr = np.ascontiguousarray(bq_.reshape(DC, P).T)
    bkr = np.ascontiguousarray(bk_.reshape(DC, P).T)

    in_maps = []
    hsT_b = {}
    for b in range(B):
        hsT_b[b] = np.ascontiguousarray(hs[b].T).astype(bf16)
    for c in range(NCORES):
        b, qh = c // 2, c % 2
        q0 = qh * QL
        m = {
            "hsT": hsT_b[b],
            "hsqT": np.ascontiguousarray(hsT_b[b][:, q0:q0 + QL]),
            "res": hs[b, q0:q0 + QL] + bo_,
            "wq": wq_, "wk": wk_, "wv": wv_, "wo": wo_,
            "bqr": bqr, "bkr": bkr,
        }
        if use_bv:
            m["bvrep"] = np.ascontiguousarray(
                np.broadcast_to(bv_, (P, D)).astype(np.float32))
        if use_mask:
            mb = (1.0 - mask[b].astype(np.float32)) * (-10000.0 * 32.0)
            m["mrep"] = np.ascontiguousarray(np.broadcast_to(mb, (P, S)))
            m["mcol"] = np.ascontiguousarray(mb.reshape(DC, P).T)
        if use_gamma:
            m["grep"] = np.ascontiguousarray(np.broadcast_to(g_, (P, D)))
        if use_beta:
            m["brep"] = np.ascontiguousarray(np.broadcast_to(be_, (P, D)))
        in_maps.append(m)
    return flags, in_maps


def kernel(**inputs):
    from concourse.bass_utils import run_bass_kernel_spmd

    flags, in_maps = _prep(**inputs)
    nc = _get_nc(flags)
    r = run_bass_kernel_spmd(nc, in_maps, core_ids=list(range(NCORES)))

    out = np.empty((B, S, D), np.float32)
    probs = np.empty((B, H, S, S), np.float32)
    for c in range(NCORES):
        b, qh = c // 2, c % 2
        q0 = qh * QL
        out[b, q0:q0 + QL] = r.results[c]["out"]
        probs[b, :, q0:q0 + QL, :] = r.results[c]["probs"]
    return out, probs


# revision 54
# speedup vs baseline: 1.2722x; 1.0146x over previous
"""ALBERT attention layer (B=4, S=1024, D=1024, H=16) on 8 Trainium2 NeuronCores.

Sharding: core c = (batch b = c//2, query-half qh = c%2). Each core computes
the attention output rows q0 = qh*512 .. q0+512 of its batch: it projects
K/V for the full batch (duplicated across the 2 cores sharing a batch --
cheaper than a collective), Q for its own rows only, and produces its slice
of both outputs (out[b, q0:q0+512, :] and probs[b, :, q0:q0+512, :]).

Layout strategy per core:
  - hidden_states arrives host-pre-transposed (hsT, [D, S], bf16) so that
    K^T/Q^T ([d_col, s]) and V ([s, d_col]) all come straight out of the
    tensor engine with no on-device transposes.
  - scores are computed twice: once as S[q, k] (softmax over the free dim,
    exp with fused row-sum accum_out -> f32 probs output), and once as
    S^T[k, q] (row-tiled K=64 matmul pairs) whose exp gives the bf16
    operand P^T for the context matmul, which directly yields ctx^T --
    exactly the lhsT the output projection needs.
  - softmax 1/sum for the ctx path is transposed once per head-pair via the
    tensor engine ([128, 8] -> [8, 128]) and broadcast across partitions
    with a tiny DRAM round-trip.
  - head pairs are software-pipelined: the S-side (probs) work of pair hp
    is emitted alongside the S^T/ctx work of pair hp-1 so ScalarE (the
    long pole: 128 exp passes) always has runway and PE never idles long
    enough for the HAM clock gate to drop it to half rate.
"""

import numpy as np
import ml_dtypes

B, S, D, H = 4, 1024, 1024, 16
DH = D // H           # 64
P = 128
QL = S // 2           # 512 query rows per core
NCORES = 2 * B        # 8
DC = D // P           # 8 column chunks
QC = QL // P          # 4 query chunks per core
HP = H // 2           # 8 head pairs
SCALE = 1.0 / 32.0    # ALBERT: 1/sqrt(hidden_size) = 1/sqrt(1024)
EPS = 1e-12

_BUILD_CACHE = {}


def _split_wide_waits(nc, mybir, max_waits=1):
    """This walrus build rejects instructions carrying more than one sync
    wait (TPB_CTRL-class ops at least; the Tile kernel-tail drain aggregates
    one per busy proc). Move excess waits onto preceding same-engine NOPs --
    semantically identical since the engine executes its stream in order."""
    n_split = 0
    for f in nc.m.functions:
        for bb in f.blocks:
            insts = list(bb.instructions)
            out = []
            changed = False
            for inst in insts:
                si = getattr(inst, "sync_info", None)
                waits = list(si.on_wait) if (si is not None and si.on_wait) else []
                if len(waits) > max_waits:
                    n_split += 1
                    changed = True
                    k = 0
                    while len(waits) - k > max_waits:
                        chunk = waits[k:k + max_waits]
                        k += max_waits
                        nop = mybir.InstNoOp(
                            name=nc.get_next_instruction_name(),
                            sync_info=mybir.SyncInfo(on_wait=chunk, on_update=[]),
                            bass_nofuse=True,
                            engine=inst.engine,
                        )
                        nc.register_instruction(nop)
                        out.append(nop)
                    si.on_wait = waits[k:]
                out.append(inst)
            if changed:
                bb.instructions = out
    return n_split


def _build(use_bv, use_mask, use_gamma, use_beta):
    import concourse.bass as bass
    import concourse.mybir as mybir
    import concourse.tile as tile
    from concourse.masks import make_identity
    from contextlib import ExitStack

    f32 = mybir.dt.float32
    bf16 = mybir.dt.bfloat16
    AF = mybir.ActivationFunctionType
    ALU = mybir.AluOpType

    nc = bass.Bass("TRN2", target_bir_lowering=False, debug=False,
                   num_devices=NCORES)

    # ---- I/O ----
    hsT = nc.dram_tensor("hsT", [D, S], bf16, kind="ExternalInput")
    hsqT = nc.dram_tensor("hsqT", [D, QL], bf16, kind="ExternalInput")
    res = nc.dram_tensor("res", [QL, D], f32, kind="ExternalInput")
    wq = nc.dram_tensor("wq", [D, D], bf16, kind="ExternalInput")
    wk = nc.dram_tensor("wk", [D, D], bf16, kind="ExternalInput")
    wv = nc.dram_tensor("wv", [D, D], bf16, kind="ExternalInput")
    wo = nc.dram_tensor("wo", [D, D], bf16, kind="ExternalInput")
    bqr = nc.dram_tensor("bqr", [P, DC], f32, kind="ExternalInput")
    bkr = nc.dram_tensor("bkr", [P, DC], f32, kind="ExternalInput")
    if use_bv:
        bvrep = nc.dram_tensor("bvrep", [P, D], f32, kind="ExternalInput")
    if use_mask:
        # 32 * (-10000) * (1 - mask[k]): row-replicated for the S[q,k] psum
        # add, and per-partition columns for the S^T[k,q] psum add.
        mrep = nc.dram_tensor("mrep", [P, S], f32, kind="ExternalInput")
        mcol = nc.dram_tensor("mcol", [P, DC], f32, kind="ExternalInput")
    if use_gamma:
        grep = nc.dram_tensor("grep", [P, D], f32, kind="ExternalInput")
    if use_beta:
        brep = nc.dram_tensor("brep", [P, D], f32, kind="ExternalInput")
    probs_o = nc.dram_tensor("probs", [H, QL, S], f32, kind="ExternalOutput")
    out_o = nc.dram_tensor("out", [QL, D], f32, kind="ExternalOutput")

    with tile.TileContext(nc) as tc:
        with ExitStack() as ctx:
            singles = ctx.enter_context(tc.tile_pool(name="singles", bufs=1))
            probsp = ctx.enter_context(tc.tile_pool(name="probsp", bufs=8))
            # ptp/invdp live OUTSIDE the stageA region: phase-2 tiles must
            # not inherit released-zone deps on stage A's last weight reads
            # (that would serialize the phase transition and cool the PE).
            # Stage-C-only tensors (wo/res/x) go into the released region
            # instead -- their late deps are harmless.
            ptp = ctx.enter_context(tc.tile_pool(name="ptp", bufs=2))
            invdp = ctx.enter_context(tc.tile_pool(name="invdp", bufs=2))
            statp = ctx.enter_context(tc.tile_pool(name="statp", bufs=4))
            # PSUM budget (16KB/partition): proj 3x1 + sp 2x2 + cx 1x1 banks
            psP = ctx.enter_context(tc.tile_pool(name="psP", bufs=3, space="PSUM"))
            psS = ctx.enter_context(tc.tile_pool(name="psS", bufs=2, space="PSUM"))
            psC = ctx.enter_context(tc.tile_pool(name="psC", bufs=1, space="PSUM"))
            dramp = ctx.enter_context(tc.tile_pool(name="dramp", bufs=1, space="DRAM"))

            def load_chunks(pool, name, src, width, defer=False):
                """Allocate chunk tiles; if defer, return the DMA thunks so
                the caller can sequence loads in dependency-priority order."""
                tiles, thunks = [], []
                for dd in range(DC):
                    t = pool.tile([P, width], bf16, tag=f"{name}{dd}",
                                  name=f"{name}{dd}")
                    lo, hi = dd * P, (dd + 1) * P
                    thunks.append(
                        lambda t=t, lo=lo, hi=hi: nc.sync.dma_start(
                            out=t, in_=src[lo:hi, :]))
                    tiles.append(t)
                if not defer:
                    for th in thunks:
                        th()
                return (tiles, thunks) if defer else tiles

            bqr_sb = singles.tile([P, DC], f32, tag="bqr")
            nc.sync.dma_start(out=bqr_sb, in_=bqr[:, :])
            bkr_sb = singles.tile([P, DC], f32, tag="bkr")
            nc.sync.dma_start(out=bkr_sb, in_=bkr[:, :])
            if use_bv:
                bv_sb = singles.tile([P, D], f32, tag="bvrep")
                nc.sync.dma_start(out=bv_sb, in_=bvrep[:, :])
            if use_mask:
                mrep_sb = singles.tile([P, S], f32, tag="mrep")
                nc.sync.dma_start(out=mrep_sb, in_=mrep[:, :])
                mcol_sb = singles.tile([P, DC], f32, tag="mcol")
                nc.sync.dma_start(out=mcol_sb, in_=mcol[:, :])
            if use_gamma:
                g_sb = singles.tile([P, D], f32, tag="grep")
                nc.sync.dma_start(out=g_sb, in_=grep[:, :])
            if use_beta:
                be_sb = singles.tile([P, D], f32, tag="brep")
                nc.sync.dma_start(out=be_sb, in_=brep[:, :])

            ident = singles.tile([P, P], f32, tag="ident")
            make_identity(nc, ident)
            eps_t = singles.tile([P, 1], f32, tag="eps")
            nc.vector.memset(eps_t, EPS)

            # per-(head, q-chunk) exp row sums: column h*QC + qc
            d_all = singles.tile([P, H * QC], f32, tag="d_all")
            inv_all = singles.tile([P, H * QC], f32, tag="inv_all")
            invd_dram = dramp.tile([H * QC, P], f32)

            def s_side(hp):
                """S[q,k] -> exp/softmax -> probs rows; leaves 1/d (row
                layout) in invd_dram for st_ctx to broadcast later."""
                h0, h1 = 2 * hp, 2 * hp + 1
                r0, r1 = slice(0, DH), slice(DH, 2 * DH)
                for qc in range(QC):
                    qsl = slice(qc * P, (qc + 1) * P)
                    sp = [psS.tile([P, S], f32, tag="sp", name=f"sp{i}")
                          for i in range(2)]
                    for half in range(2):
                        ks = slice(half * 512, (half + 1) * 512)
                        nc.tensor.matmul(sp[0][:, ks], qT_sb[hp][r0, qsl],
                                         kT_sb[hp][r0, ks])
                        nc.tensor.matmul(sp[1][:, ks], qT_sb[hp][r1, qsl],
                                         kT_sb[hp][r1, ks])
                    # d_all column layout: hp*8 + qc*2 + head-parity (so the
                    # two heads' sums sit adjacent for one batched reciprocal,
                    # and the per-hp transpose slice stays contiguous)
                    col0 = hp * 2 * QC + qc * 2
                    pts_ = []
                    for i, h in ((0, h0), (1, h1)):
                        if use_mask:
                            nc.vector.tensor_tensor(out=sp[i], in0=sp[i],
                                                    in1=mrep_sb, op=ALU.add)
                        pt = probsp.tile([P, S], f32, tag="probs", name="probs")
                        nc.scalar.activation(
                            out=pt, in_=sp[i], func=AF.Exp, scale=SCALE,
                            accum_out=d_all[:, col0 + i:col0 + i + 1],
                        )
                        pts_.append(pt)
                    nc.vector.reciprocal(out=inv_all[:, col0:col0 + 2],
                                         in_=d_all[:, col0:col0 + 2])
                    for i, h in ((0, h0), (1, h1)):
                        nc.vector.tensor_scalar_mul(
                            out=pts_[i], in0=pts_[i],
                            scalar1=inv_all[:, col0 + i:col0 + i + 1])
                        nc.sync.dma_start(out=probs_o[h, qsl, :], in_=pts_[i])

                # 1/d transposed to rows -> DRAM (broadcast read in st_ctx)
                dts = psC.tile([P, QL], f32, tag="cx", name="dts")
                nc.tensor.transpose(dts[0:2 * QC, 0:P],
                                    d_all[:, hp * 2 * QC:(hp + 1) * 2 * QC],
                                    ident)
                invdt = statp.tile([2 * QC, P], f32, tag="invdt")
                nc.vector.reciprocal(out=invdt, in_=dts[0:2 * QC, 0:P])
                nc.sync.dma_start(
                    out=invd_dram[hp * 2 * QC:(hp + 1) * 2 * QC, :], in_=invdt)

            # ---- stage A: projections, with the probs (S-side) work of
            # head pair c-1 interleaved so ScalarE fills during stage A ----
            # kT_sb[c]: K^T cols chunk c -> [d_col 128, s 1024]
            # qT_sb[c]: Q^T              -> [d_col 128, q 512]
            # v_sb[c]:  V rows chunk c   -> [s 128, d_col 1024]
            kT_sb, qT_sb, v_sb = [], [], []
            with tc.tile_pool(name="stageA", bufs=1) as sap:
                # first projection group (kps c=0) needs all of wk+hsT, so
                # those loads go first, interleaved to spread across queues
                wk_sb, wk_th = load_chunks(sap, "wk", wk, D, defer=True)
                hsT_sb, hsT_th = load_chunks(sap, "hsT", hsT, S, defer=True)
                wq_sb, wq_th = load_chunks(sap, "wq", wq, D, defer=True)
                hsqT_sb, hsqT_th = load_chunks(sap, "hsqT", hsqT, QL,
                                               defer=True)
                wv_sb, wv_th = load_chunks(sap, "wv", wv, D, defer=True)
                for dd in range(DC):
                    hsT_th[dd]()
                    wk_th[dd]()
                for dd in range(DC):
                    wq_th[dd]()
                    hsqT_th[dd]()
                for dd in range(DC):
                    wv_th[dd]()


                for c in range(DC):
                    kt = singles.tile([P, S], bf16, tag=f"kT{c}", name=f"kT{c}")
                    for half in range(2):
                        sl = slice(half * 512, (half + 1) * 512)
                        kps = psP.tile([P, 512], f32, tag="proj", name="kps")
                        for dd in range(DC):
                            nc.tensor.matmul(
                                kps,
                                wk_sb[dd][:, c * P:(c + 1) * P],
                                hsT_sb[dd][:, sl],
                                start=(dd == 0), stop=(dd == DC - 1),
                            )
                        nc.vector.tensor_scalar_add(out=kt[:, sl], in0=kps,
                                                    scalar1=bkr_sb[:, c:c + 1])
                    kT_sb.append(kt)

                    qps = psP.tile([P, QL], f32, tag="proj", name="qps")
                    for dd in range(DC):
                        nc.tensor.matmul(
                            qps, wq_sb[dd][:, c * P:(c + 1) * P], hsqT_sb[dd],
                            start=(dd == 0), stop=(dd == DC - 1),
                        )
                    qt = singles.tile([P, QL], bf16, tag=f"qT{c}", name=f"qT{c}")
                    nc.vector.tensor_scalar_add(out=qt, in0=qps,
                                                scalar1=bqr_sb[:, c:c + 1])
                    qT_sb.append(qt)

                    # attention S-side for this very chunk: keeps ScalarE fed
                    # throughout stage A
                    s_side(c)

                # V projections last: V is only needed by the phase-2 ctx
                # matmuls, and this dense block keeps PE hot (HAM at full
                # clock) exactly while ScalarE catches up on the s_side
                # exp backlog.
                for c in range(DC):
                    vt = singles.tile([P, D], bf16, tag=f"v{c}", name=f"v{c}")
                    for half in range(2):
                        sl = slice(half * 512, (half + 1) * 512)
                        vps = psP.tile([P, 512], f32, tag="proj", name="vps")
                        for dd in range(DC):
                            nc.tensor.matmul(
                                vps,
                                hsT_sb[dd][:, c * P:(c + 1) * P],
                                wv_sb[dd][:, sl],
                                start=(dd == 0), stop=(dd == DC - 1),
                            )
                        if use_bv:
                            nc.vector.tensor_tensor(out=vt[:, sl], in0=vps,
                                                    in1=bv_sb[:, sl],
                                                    op=ALU.add)
                        else:
                            nc.vector.tensor_copy(out=vt[:, sl], in_=vps)
                    v_sb.append(vt)

            # stage-C tensors reuse the space stage A released
            latep = ctx.enter_context(tc.tile_pool(name="latep", bufs=1))
            xp = ctx.enter_context(tc.tile_pool(name="xp", bufs=2))
            wo_sb = load_chunks(latep, "wo", wo, D)
            # residual (+bo) rows and SBUF accumulators for the output
            # projection, which runs incrementally (one dd-pair batch after
            # every second head pair) on the otherwise-idle psP slots, so
            # only the last quarter of it remains in the stage-C tail
            res_sb, oacc = [], []
            for qc in range(QC):
                rt = latep.tile([P, D], f32, tag=f"res{qc}", name=f"res{qc}")
                nc.sync.dma_start(out=rt, in_=res[qc * P:(qc + 1) * P, :])
                res_sb.append(rt)
                oacc.append(latep.tile([P, D], f32, tag=f"oacc{qc}",
                                       name=f"oacc{qc}"))

            def st_ctx(hp):
                """S^T -> exp -> P^T; ctx^T = V^T @ P^T, scaled by 1/d."""
                invd_rep = invdp.tile([P, QC, P], f32, tag="invd_rep",
                                      name="invd_rep")
                for par in range(2):
                    src = bass.AP(
                        tensor=invd_dram.tensor,
                        offset=invd_dram.offset + (hp * 2 * QC + par) * P,
                        ap=[[0, DH], [2 * P, QC], [1, P]],
                    )
                    nc.gpsimd.dma_start(out=invd_rep[par * DH:(par + 1) * DH],
                                        in_=src)
                h0, h1 = 2 * hp, 2 * hp + 1
                r0, r1 = slice(0, DH), slice(DH, 2 * DH)
                cxs = psC.tile([P, QL], f32, tag="cx", name="cxs")
                pts = {}

                def ctx_mms(kc):
                    # PE is in-order: emitting ctx(kc) right after exp(kc)
                    # would make PE block on ScalarE each step. Lag by 2 so
                    # the exp is long done when PE reaches the ctx matmuls.
                    pt2 = pts.pop(kc)
                    nc.tensor.matmul(cxs[0:DH, :],
                                     v_sb[kc][:, h0 * DH:(h0 + 1) * DH],
                                     pt2[:, 0:QL],
                                     start=(kc == 0), stop=(kc == DC - 1))
                    nc.tensor.matmul(cxs[DH:2 * DH, :],
                                     v_sb[kc][:, h1 * DH:(h1 + 1) * DH],
                                     pt2[:, QL:2 * QL],
                                     start=(kc == 0), stop=(kc == DC - 1))

                for kc in range(DC):
                    ksl = slice(kc * P, (kc + 1) * P)
                    st = psS.tile([P, 2 * QL], f32, tag="sp", name="st")
                    nc.tensor.matmul(st[:, 0:QL], kT_sb[hp][r0, ksl],
                                     qT_sb[hp][r0, :])
                    nc.tensor.matmul(st[:, QL:2 * QL], kT_sb[hp][r1, ksl],
                                     qT_sb[hp][r1, :])
                    if use_mask:
                        nc.vector.tensor_scalar_add(
                            out=st, in0=st, scalar1=mcol_sb[:, kc:kc + 1])
                    pt2 = ptp.tile([P, 2 * QL], bf16, tag=f"pt{kc}",
                                   name=f"pt{kc}")
                    nc.scalar.activation(out=pt2, in_=st, func=AF.Exp,
                                         scale=SCALE)
                    pts[kc] = pt2
                    if kc >= 2:
                        ctx_mms(kc - 2)
                ctx_mms(DC - 2)
                ctx_mms(DC - 1)
                cxt = singles.tile([P, QL], bf16, tag=f"ctxT{hp}",
                                   name=f"ctxT{hp}")
                nc.vector.tensor_tensor(
                    out=cxt, in0=cxs,
                    in1=invd_rep.rearrange("p a b -> p (a b)"), op=ALU.mult)
                return cxt

            ctxT_sb = []
            for hp in range(HP):
                ctxT_sb.append(st_ctx(hp))
                if hp % 2 == 1:
                    batch = hp // 2
                    for qc in range(QC):
                        qsl_ = slice(qc * P, (qc + 1) * P)
                        for half in range(2):
                            sl = slice(half * 512, (half + 1) * 512)
                            pps = psP.tile([P, 512], f32, tag="proj",
                                           name="pps")
                            for j, dd in enumerate((2 * batch, 2 * batch + 1)):
                                nc.tensor.matmul(
                                    pps, ctxT_sb[dd][:, qsl_],
                                    wo_sb[dd][:, sl],
                                    start=(j == 0), stop=(j == 1))
                            if batch == 0:
                                nc.vector.tensor_tensor(
                                    out=oacc[qc][:, sl], in0=pps,
                                    in1=res_sb[qc][:, sl], op=ALU.add)
                            else:
                                nc.vector.tensor_tensor(
                                    out=oacc[qc][:, sl],
                                    in0=oacc[qc][:, sl], in1=pps,
                                    op=ALU.add)

            # ---- stage C: LayerNorm epilogue (out-proj already folded) ----
            for qc in range(QC):
                qsl = slice(qc * P, (qc + 1) * P)
                x = oacc[qc]
                stats = statp.tile([P, 2, 6], f32, tag="stats", name="stats")
                for i in range(2):
                    nc.vector.bn_stats(out=stats[:, i, :],
                                       in_=x[:, i * 512:(i + 1) * 512])
                mv = statp.tile([P, 2], f32, tag="mv", name="mv")
                nc.vector.bn_aggr(out=mv, in_=stats)
                std = statp.tile([P, 1], f32, tag="std", name="std")
                nc.scalar.activation(out=std, in_=mv[:, 1:2], func=AF.Sqrt,
                                     bias=eps_t, scale=1.0)
                rstd = statp.tile([P, 1], f32, tag="rstd", name="rstd")
                nc.vector.reciprocal(out=rstd, in_=std)
                ot = xp.tile([P, D], f32, tag="ot", name="ot")
                nc.vector.tensor_scalar(
                    out=ot, in0=x, scalar1=mv[:, 0:1], scalar2=rstd,
                    op0=ALU.subtract, op1=ALU.mult,
                )
                if use_gamma:
                    nc.vector.tensor_tensor(out=ot, in0=ot, in1=g_sb,
                                            op=ALU.mult)
                if use_beta:
                    nc.vector.tensor_tensor(out=ot, in0=ot, in1=be_sb,
                                            op=ALU.add)
                nc.sync.dma_start(out=out_o[qsl, :], in_=ot)

    _split_wide_waits(nc, mybir)
    return nc


def _get_nc(flags):
    if flags not in _BUILD_CACHE:
        _BUILD_CACHE[flags] = _build(*flags)
    return _BUILD_CACHE[flags]


def _prep(hidden_states, attention_mask, wq, bq, wk, bk, wv, bv, wo, bo,
          ln_gamma, ln_beta):
    """Returns (flags, in_maps)."""
    bf16 = ml_dtypes.bfloat16
    hs = np.asarray(hidden_states, dtype=np.float32)
    mask = np.asarray(attention_mask)
    wq_ = np.asarray(wq, dtype=np.float32).astype(bf16)
    wk_ = np.asarray(wk, dtype=np.float32).astype(bf16)
    wv_ = np.asarray(wv, dtype=np.float32).astype(bf16)
    wo_ = np.asarray(wo, dtype=np.float32).astype(bf16)
    bq_ = np.asarray(bq, dtype=np.float32)
    bk_ = np.asarray(bk, dtype=np.float32)
    bv_ = np.asarray(bv, dtype=np.float32)
    bo_ = np.asarray(bo, dtype=np.float32)
    g_ = np.asarray(ln_gamma, dtype=np.float32)
    be_ = np.asarray(ln_beta, dtype=np.float32)

    use_bv = bool(np.any(bv_ != 0.0))
    use_mask = bool(np.any(mask != 1))
    use_gamma = bool(np.any(g_ != 1.0))
    use_beta = bool(np.any(be_ != 0.0))
    flags = (use_bv, use_mask, use_gamma, use_beta)

    bqr = np.ascontiguousarray(bq_.reshape(DC, P).T)
    bkr = np.ascontiguousarray(bk_.reshape(DC, P).T)

    in_maps = []
    hsT_b = {}
    for b in range(B):
        hsT_b[b] = np.ascontiguousarray(hs[b].T).astype(bf16)
    for c in range(NCORES):
        b, qh = c // 2, c % 2
        q0 = qh * QL
        m = {
            "hsT": hsT_b[b],
            "hsqT": np.ascontiguousarray(hsT_b[b][:, q0:q0 + QL]),
            "res": hs[b, q0:q0 + QL] + bo_,
            "wq": wq_, "wk": wk_, "wv": wv_, "wo": wo_,
            "bqr": bqr, "bkr": bkr,
        }
        if use_bv:
            m["bvrep"] = np.ascontiguousarray(
                np.broadcast_to(bv_, (P, D)).astype(np.float32))
        if use_mask:
            mb = (1.0 - mask[b].astype(np.float32)) * (-10000.0 * 32.0)
            m["mrep"] = np.ascontiguousarray(np.broadcast_to(mb, (P, S)))
            m["mcol"] = np.ascontiguousarray(mb.reshape(DC, P).T)
        if use_gamma:
            m["grep"] = np.ascontiguousarray(np.broadcast_to(g_, (P, D)))
        if use_beta:
            m["brep"] = np.ascontiguousarray(np.broadcast_to(be_, (P, D)))
        in_maps.append(m)
    return flags, in_maps


def kernel(**inputs):
    from concourse.bass_utils import run_bass_kernel_spmd

    flags, in_maps = _prep(**inputs)
    nc = _get_nc(flags)
    r = run_bass_kernel_spmd(nc, in_maps, core_ids=list(range(NCORES)))

    out = np.empty((B, S, D), np.float32)
    probs = np.empty((B, H, S, S), np.float32)
    for c in range(NCORES):
        b, qh = c // 2, c % 2
        q0 = qh * QL
        out[b, q0:q0 + QL] = r.results[c]["out"]
        probs[b, :, q0:q0 + QL, :] = r.results[c]["probs"]
    return out, probs


# revision 55
# speedup vs baseline: 1.2779x; 1.0045x over previous
"""ALBERT attention layer (B=4, S=1024, D=1024, H=16) on 8 Trainium2 NeuronCores.

Sharding: core c = (batch b = c//2, query-half qh = c%2). Each core computes
the attention output rows q0 = qh*512 .. q0+512 of its batch: it projects
K/V for the full batch (duplicated across the 2 cores sharing a batch --
cheaper than a collective), Q for its own rows only, and produces its slice
of both outputs (out[b, q0:q0+512, :] and probs[b, :, q0:q0+512, :]).

Layout strategy per core:
  - hidden_states arrives host-pre-transposed (hsT, [D, S], bf16) so that
    K^T/Q^T ([d_col, s]) and V ([s, d_col]) all come straight out of the
    tensor engine with no on-device transposes.
  - scores are computed twice: once as S[q, k] (softmax over the free dim,
    exp with fused row-sum accum_out -> f32 probs output), and once as
    S^T[k, q] (row-tiled K=64 matmul pairs) whose exp gives the bf16
    operand P^T for the context matmul, which directly yields ctx^T --
    exactly the lhsT the output projection needs.
  - softmax 1/sum for the ctx path is transposed once per head-pair via the
    tensor engine ([128, 8] -> [8, 128]) and broadcast across partitions
    with a tiny DRAM round-trip.
  - head pairs are software-pipelined: the S-side (probs) work of pair hp
    is emitted alongside the S^T/ctx work of pair hp-1 so ScalarE (the
    long pole: 128 exp passes) always has runway and PE never idles long
    enough for the HAM clock gate to drop it to half rate.
"""

import numpy as np
import ml_dtypes

B, S, D, H = 4, 1024, 1024, 16
DH = D // H           # 64
P = 128
QL = S // 2           # 512 query rows per core
NCORES = 2 * B        # 8
DC = D // P           # 8 column chunks
QC = QL // P          # 4 query chunks per core
HP = H // 2           # 8 head pairs
SCALE = 1.0 / 32.0    # ALBERT: 1/sqrt(hidden_size) = 1/sqrt(1024)
EPS = 1e-12

_BUILD_CACHE = {}


def _split_wide_waits(nc, mybir, max_waits=1):
    """This walrus build rejects instructions carrying more than one sync
    wait (TPB_CTRL-class ops at least; the Tile kernel-tail drain aggregates
    one per busy proc). Move excess waits onto preceding same-engine NOPs --
    semantically identical since the engine executes its stream in order."""
    n_split = 0
    for f in nc.m.functions:
        for bb in f.blocks:
            insts = list(bb.instructions)
            out = []
            changed = False
            for inst in insts:
                si = getattr(inst, "sync_info", None)
                waits = list(si.on_wait) if (si is not None and si.on_wait) else []
                if len(waits) > max_waits:
                    n_split += 1
                    changed = True
                    k = 0
                    while len(waits) - k > max_waits:
                        chunk = waits[k:k + max_waits]
                        k += max_waits
                        nop = mybir.InstNoOp(
                            name=nc.get_next_instruction_name(),
                            sync_info=mybir.SyncInfo(on_wait=chunk, on_update=[]),
                            bass_nofuse=True,
                            engine=inst.engine,
                        )
                        nc.register_instruction(nop)
                        out.append(nop)
                    si.on_wait = waits[k:]
                out.append(inst)
            if changed:
                bb.instructions = out
    return n_split


def _build(use_bv, use_mask, use_gamma, use_beta):
    import concourse.bass as bass
    import concourse.mybir as mybir
    import concourse.tile as tile
    from concourse.masks import make_identity
    from contextlib import ExitStack

    f32 = mybir.dt.float32
    bf16 = mybir.dt.bfloat16
    AF = mybir.ActivationFunctionType
    ALU = mybir.AluOpType

    nc = bass.Bass("TRN2", target_bir_lowering=False, debug=False,
                   num_devices=NCORES)

    # ---- I/O ----
    hsT = nc.dram_tensor("hsT", [D, S], bf16, kind="ExternalInput")
    hsqT = nc.dram_tensor("hsqT", [D, QL], bf16, kind="ExternalInput")
    res = nc.dram_tensor("res", [QL, D], f32, kind="ExternalInput")
    wq = nc.dram_tensor("wq", [D, D], bf16, kind="ExternalInput")
    wk = nc.dram_tensor("wk", [D, D], bf16, kind="ExternalInput")
    wv = nc.dram_tensor("wv", [D, D], bf16, kind="ExternalInput")
    wo = nc.dram_tensor("wo", [D, D], bf16, kind="ExternalInput")
    bqr = nc.dram_tensor("bqr", [P, DC], f32, kind="ExternalInput")
    bkr = nc.dram_tensor("bkr", [P, DC], f32, kind="ExternalInput")
    if use_bv:
        bvrep = nc.dram_tensor("bvrep", [P, D], f32, kind="ExternalInput")
    if use_mask:
        # 32 * (-10000) * (1 - mask[k]): row-replicated for the S[q,k] psum
        # add, and per-partition columns for the S^T[k,q] psum add.
        mrep = nc.dram_tensor("mrep", [P, S], f32, kind="ExternalInput")
        mcol = nc.dram_tensor("mcol", [P, DC], f32, kind="ExternalInput")
    if use_gamma:
        grep = nc.dram_tensor("grep", [P, D], f32, kind="ExternalInput")
    if use_beta:
        brep = nc.dram_tensor("brep", [P, D], f32, kind="ExternalInput")
    probs_o = nc.dram_tensor("probs", [H, QL, S], f32, kind="ExternalOutput")
    out_o = nc.dram_tensor("out", [QL, D], f32, kind="ExternalOutput")

    with tile.TileContext(nc) as tc:
        with ExitStack() as ctx:
            singles = ctx.enter_context(tc.tile_pool(name="singles", bufs=1))
            probsp = ctx.enter_context(tc.tile_pool(name="probsp", bufs=9))
            # ptp/invdp live OUTSIDE the stageA region: phase-2 tiles must
            # not inherit released-zone deps on stage A's last weight reads
            # (that would serialize the phase transition and cool the PE).
            # Stage-C-only tensors (wo/res/x) go into the released region
            # instead -- their late deps are harmless.
            ptp = ctx.enter_context(tc.tile_pool(name="ptp", bufs=2))
            invdp = ctx.enter_context(tc.tile_pool(name="invdp", bufs=2))
            statp = ctx.enter_context(tc.tile_pool(name="statp", bufs=4))
            # PSUM budget (16KB/partition): proj 3x1 + sp 2x2 + cx 1x1 banks
            psP = ctx.enter_context(tc.tile_pool(name="psP", bufs=3, space="PSUM"))
            psS = ctx.enter_context(tc.tile_pool(name="psS", bufs=2, space="PSUM"))
            psC = ctx.enter_context(tc.tile_pool(name="psC", bufs=1, space="PSUM"))
            dramp = ctx.enter_context(tc.tile_pool(name="dramp", bufs=1, space="DRAM"))

            def load_chunks(pool, name, src, width, defer=False):
                """Allocate chunk tiles; if defer, return the DMA thunks so
                the caller can sequence loads in dependency-priority order."""
                tiles, thunks = [], []
                for dd in range(DC):
                    t = pool.tile([P, width], bf16, tag=f"{name}{dd}",
                                  name=f"{name}{dd}")
                    lo, hi = dd * P, (dd + 1) * P
                    thunks.append(
                        lambda t=t, lo=lo, hi=hi: nc.sync.dma_start(
                            out=t, in_=src[lo:hi, :]))
                    tiles.append(t)
                if not defer:
                    for th in thunks:
                        th()
                return (tiles, thunks) if defer else tiles

            bqr_sb = singles.tile([P, DC], f32, tag="bqr")
            nc.sync.dma_start(out=bqr_sb, in_=bqr[:, :])
            bkr_sb = singles.tile([P, DC], f32, tag="bkr")
            nc.sync.dma_start(out=bkr_sb, in_=bkr[:, :])
            if use_bv:
                bv_sb = singles.tile([P, D], f32, tag="bvrep")
                nc.sync.dma_start(out=bv_sb, in_=bvrep[:, :])
            if use_mask:
                mrep_sb = singles.tile([P, S], f32, tag="mrep")
                nc.sync.dma_start(out=mrep_sb, in_=mrep[:, :])
                mcol_sb = singles.tile([P, DC], f32, tag="mcol")
                nc.sync.dma_start(out=mcol_sb, in_=mcol[:, :])
            if use_gamma:
                g_sb = singles.tile([P, D], f32, tag="grep")
                nc.sync.dma_start(out=g_sb, in_=grep[:, :])
            if use_beta:
                be_sb = singles.tile([P, D], f32, tag="brep")
                nc.sync.dma_start(out=be_sb, in_=brep[:, :])

            ident = singles.tile([P, P], f32, tag="ident")
            make_identity(nc, ident)
            eps_t = singles.tile([P, 1], f32, tag="eps")
            nc.vector.memset(eps_t, EPS)

            # per-(head, q-chunk) exp row sums: column h*QC + qc
            d_all = singles.tile([P, H * QC], f32, tag="d_all")
            inv_all = singles.tile([P, H * QC], f32, tag="inv_all")
            invd_dram = dramp.tile([H * QC, P], f32)

            def s_side(hp):
                """S[q,k] -> exp/softmax -> probs rows; leaves 1/d (row
                layout) in invd_dram for st_ctx to broadcast later."""
                h0, h1 = 2 * hp, 2 * hp + 1
                r0, r1 = slice(0, DH), slice(DH, 2 * DH)
                for qc in range(QC):
                    qsl = slice(qc * P, (qc + 1) * P)
                    sp = [psS.tile([P, S], f32, tag="sp", name=f"sp{i}")
                          for i in range(2)]
                    for half in range(2):
                        ks = slice(half * 512, (half + 1) * 512)
                        nc.tensor.matmul(sp[0][:, ks], qT_sb[hp][r0, qsl],
                                         kT_sb[hp][r0, ks])
                        nc.tensor.matmul(sp[1][:, ks], qT_sb[hp][r1, qsl],
                                         kT_sb[hp][r1, ks])
                    # d_all column layout: hp*8 + qc*2 + head-parity (so the
                    # two heads' sums sit adjacent for one batched reciprocal,
                    # and the per-hp transpose slice stays contiguous)
                    col0 = hp * 2 * QC + qc * 2
                    pts_ = []
                    for i, h in ((0, h0), (1, h1)):
                        if use_mask:
                            nc.vector.tensor_tensor(out=sp[i], in0=sp[i],
                                                    in1=mrep_sb, op=ALU.add)
                        pt = probsp.tile([P, S], f32, tag="probs", name="probs")
                        nc.scalar.activation(
                            out=pt, in_=sp[i], func=AF.Exp, scale=SCALE,
                            accum_out=d_all[:, col0 + i:col0 + i + 1],
                        )
                        pts_.append(pt)
                    nc.vector.reciprocal(out=inv_all[:, col0:col0 + 2],
                                         in_=d_all[:, col0:col0 + 2])
                    for i, h in ((0, h0), (1, h1)):
                        nc.vector.tensor_scalar_mul(
                            out=pts_[i], in0=pts_[i],
                            scalar1=inv_all[:, col0 + i:col0 + i + 1])
                        nc.sync.dma_start(out=probs_o[h, qsl, :], in_=pts_[i])

                # 1/d transposed to rows -> DRAM (broadcast read in st_ctx)
                dts = psC.tile([P, QL], f32, tag="cx", name="dts")
                nc.tensor.transpose(dts[0:2 * QC, 0:P],
                                    d_all[:, hp * 2 * QC:(hp + 1) * 2 * QC],
                                    ident)
                invdt = statp.tile([2 * QC, P], f32, tag="invdt")
                nc.vector.reciprocal(out=invdt, in_=dts[0:2 * QC, 0:P])
                nc.sync.dma_start(
                    out=invd_dram[hp * 2 * QC:(hp + 1) * 2 * QC, :], in_=invdt)

            # ---- stage A: projections, with the probs (S-side) work of
            # head pair c-1 interleaved so ScalarE fills during stage A ----
            # kT_sb[c]: K^T cols chunk c -> [d_col 128, s 1024]
            # qT_sb[c]: Q^T              -> [d_col 128, q 512]
            # v_sb[c]:  V rows chunk c   -> [s 128, d_col 1024]
            kT_sb, qT_sb, v_sb = [], [], []
            with tc.tile_pool(name="stageA", bufs=1) as sap:
                # first projection group (kps c=0) needs all of wk+hsT, so
                # those loads go first, interleaved to spread across queues
                wk_sb, wk_th = load_chunks(sap, "wk", wk, D, defer=True)
                hsT_sb, hsT_th = load_chunks(sap, "hsT", hsT, S, defer=True)
                wq_sb, wq_th = load_chunks(sap, "wq", wq, D, defer=True)
                hsqT_sb, hsqT_th = load_chunks(sap, "hsqT", hsqT, QL,
                                               defer=True)
                wv_sb, wv_th = load_chunks(sap, "wv", wv, D, defer=True)
                for dd in range(DC):
                    hsT_th[dd]()
                    wk_th[dd]()
                for dd in range(DC):
                    wq_th[dd]()
                    hsqT_th[dd]()
                for dd in range(DC):
                    wv_th[dd]()


                for c in range(DC):
                    kt = singles.tile([P, S], bf16, tag=f"kT{c}", name=f"kT{c}")
                    for half in range(2):
                        sl = slice(half * 512, (half + 1) * 512)
                        kps = psP.tile([P, 512], f32, tag="proj", name="kps")
                        for dd in range(DC):
                            nc.tensor.matmul(
                                kps,
                                wk_sb[dd][:, c * P:(c + 1) * P],
                                hsT_sb[dd][:, sl],
                                start=(dd == 0), stop=(dd == DC - 1),
                            )
                        nc.vector.tensor_scalar_add(out=kt[:, sl], in0=kps,
                                                    scalar1=bkr_sb[:, c:c + 1])
                    kT_sb.append(kt)

                    qps = psP.tile([P, QL], f32, tag="proj", name="qps")
                    for dd in range(DC):
                        nc.tensor.matmul(
                            qps, wq_sb[dd][:, c * P:(c + 1) * P], hsqT_sb[dd],
                            start=(dd == 0), stop=(dd == DC - 1),
                        )
                    qt = singles.tile([P, QL], bf16, tag=f"qT{c}", name=f"qT{c}")
                    nc.vector.tensor_scalar_add(out=qt, in0=qps,
                                                scalar1=bqr_sb[:, c:c + 1])
                    qT_sb.append(qt)

                    # attention S-side for this very chunk: keeps ScalarE fed
                    # throughout stage A
                    s_side(c)

                # V projections last: V is only needed by the phase-2 ctx
                # matmuls, and this dense block keeps PE hot (HAM at full
                # clock) exactly while ScalarE catches up on the s_side
                # exp backlog.
                for c in range(DC):
                    vt = singles.tile([P, D], bf16, tag=f"v{c}", name=f"v{c}")
                    for half in range(2):
                        sl = slice(half * 512, (half + 1) * 512)
                        vps = psP.tile([P, 512], f32, tag="proj", name="vps")
                        for dd in range(DC):
                            nc.tensor.matmul(
                                vps,
                                hsT_sb[dd][:, c * P:(c + 1) * P],
                                wv_sb[dd][:, sl],
                                start=(dd == 0), stop=(dd == DC - 1),
                            )
                        if use_bv:
                            nc.vector.tensor_tensor(out=vt[:, sl], in0=vps,
                                                    in1=bv_sb[:, sl],
                                                    op=ALU.add)
                        else:
                            nc.vector.tensor_copy(out=vt[:, sl], in_=vps)
                    v_sb.append(vt)

            # stage-C tensors reuse the space stage A released
            latep = ctx.enter_context(tc.tile_pool(name="latep", bufs=1))
            xp = ctx.enter_context(tc.tile_pool(name="xp", bufs=2))
            wo_sb = load_chunks(latep, "wo", wo, D)
            # residual (+bo) rows and SBUF accumulators for the output
            # projection, which runs incrementally (one dd-pair batch after
            # every second head pair) on the otherwise-idle psP slots, so
            # only the last quarter of it remains in the stage-C tail
            res_sb, oacc = [], []
            for qc in range(QC):
                rt = latep.tile([P, D], f32, tag=f"res{qc}", name=f"res{qc}")
                nc.sync.dma_start(out=rt, in_=res[qc * P:(qc + 1) * P, :])
                res_sb.append(rt)
                oacc.append(latep.tile([P, D], f32, tag=f"oacc{qc}",
                                       name=f"oacc{qc}"))

            def st_ctx(hp):
                """S^T -> exp -> P^T; ctx^T = V^T @ P^T, scaled by 1/d."""
                invd_rep = invdp.tile([P, QC, P], f32, tag="invd_rep",
                                      name="invd_rep")
                for par in range(2):
                    src = bass.AP(
                        tensor=invd_dram.tensor,
                        offset=invd_dram.offset + (hp * 2 * QC + par) * P,
                        ap=[[0, DH], [2 * P, QC], [1, P]],
                    )
                    nc.gpsimd.dma_start(out=invd_rep[par * DH:(par + 1) * DH],
                                        in_=src)
                h0, h1 = 2 * hp, 2 * hp + 1
                r0, r1 = slice(0, DH), slice(DH, 2 * DH)
                cxs = psC.tile([P, QL], f32, tag="cx", name="cxs")
                pts = {}

                def ctx_mms(kc):
                    # PE is in-order: emitting ctx(kc) right after exp(kc)
                    # would make PE block on ScalarE each step. Lag by 2 so
                    # the exp is long done when PE reaches the ctx matmuls.
                    pt2 = pts.pop(kc)
                    nc.tensor.matmul(cxs[0:DH, :],
                                     v_sb[kc][:, h0 * DH:(h0 + 1) * DH],
                                     pt2[:, 0:QL],
                                     start=(kc == 0), stop=(kc == DC - 1))
                    nc.tensor.matmul(cxs[DH:2 * DH, :],
                                     v_sb[kc][:, h1 * DH:(h1 + 1) * DH],
                                     pt2[:, QL:2 * QL],
                                     start=(kc == 0), stop=(kc == DC - 1))

                for kc in range(DC):
                    ksl = slice(kc * P, (kc + 1) * P)
                    st = psS.tile([P, 2 * QL], f32, tag="sp", name="st")
                    nc.tensor.matmul(st[:, 0:QL], kT_sb[hp][r0, ksl],
                                     qT_sb[hp][r0, :])
                    nc.tensor.matmul(st[:, QL:2 * QL], kT_sb[hp][r1, ksl],
                                     qT_sb[hp][r1, :])
                    if use_mask:
                        nc.vector.tensor_scalar_add(
                            out=st, in0=st, scalar1=mcol_sb[:, kc:kc + 1])
                    pt2 = ptp.tile([P, 2 * QL], bf16, tag=f"pt{kc}",
                                   name=f"pt{kc}")
                    nc.scalar.activation(out=pt2, in_=st, func=AF.Exp,
                                         scale=SCALE)
                    pts[kc] = pt2
                    if kc >= 2:
                        ctx_mms(kc - 2)
                ctx_mms(DC - 2)
                ctx_mms(DC - 1)
                cxt = singles.tile([P, QL], bf16, tag=f"ctxT{hp}",
                                   name=f"ctxT{hp}")
                nc.vector.tensor_tensor(
                    out=cxt, in0=cxs,
                    in1=invd_rep.rearrange("p a b -> p (a b)"), op=ALU.mult)
                return cxt

            ctxT_sb = []
            for hp in range(HP):
                ctxT_sb.append(st_ctx(hp))
                if hp % 2 == 1:
                    batch = hp // 2
                    for qc in range(QC):
                        qsl_ = slice(qc * P, (qc + 1) * P)
                        for half in range(2):
                            sl = slice(half * 512, (half + 1) * 512)
                            pps = psP.tile([P, 512], f32, tag="proj",
                                           name="pps")
                            for j, dd in enumerate((2 * batch, 2 * batch + 1)):
                                nc.tensor.matmul(
                                    pps, ctxT_sb[dd][:, qsl_],
                                    wo_sb[dd][:, sl],
                                    start=(j == 0), stop=(j == 1))
                            if batch == 0:
                                nc.vector.tensor_tensor(
                                    out=oacc[qc][:, sl], in0=pps,
                                    in1=res_sb[qc][:, sl], op=ALU.add)
                            else:
                                nc.vector.tensor_tensor(
                                    out=oacc[qc][:, sl],
                                    in0=oacc[qc][:, sl], in1=pps,
                                    op=ALU.add)

            # ---- stage C: LayerNorm epilogue (out-proj already folded) ----
            for qc in range(QC):
                qsl = slice(qc * P, (qc + 1) * P)
                x = oacc[qc]
                stats = statp.tile([P, 2, 6], f32, tag="stats", name="stats")
                for i in range(2):
                    nc.vector.bn_stats(out=stats[:, i, :],
                                       in_=x[:, i * 512:(i + 1) * 512])
                mv = statp.tile([P, 2], f32, tag="mv", name="mv")
                nc.vector.bn_aggr(out=mv, in_=stats)
                std = statp.tile([P, 1], f32, tag="std", name="std")
                nc.scalar.activation(out=std, in_=mv[:, 1:2], func=AF.Sqrt,
                                     bias=eps_t, scale=1.0)
                rstd = statp.tile([P, 1], f32, tag="rstd", name="rstd")
                nc.vector.reciprocal(out=rstd, in_=std)
                ot = xp.tile([P, D], f32, tag="ot", name="ot")
                nc.vector.tensor_scalar(
                    out=ot, in0=x, scalar1=mv[:, 0:1], scalar2=rstd,
                    op0=ALU.subtract, op1=ALU.mult,
                )
                if use_gamma:
                    nc.vector.tensor_tensor(out=ot, in0=ot, in1=g_sb,
                                            op=ALU.mult)
                if use_beta:
                    nc.vector.tensor_tensor(out=ot, in0=ot, in1=be_sb,
                                            op=ALU.add)
                nc.sync.dma_start(out=out_o[qsl, :], in_=ot)

    _split_wide_waits(nc, mybir)
    return nc


def _get_nc(flags):
    if flags not in _BUILD_CACHE:
        _BUILD_CACHE[flags] = _build(*flags)
    return _BUILD_CACHE[flags]


def _prep(hidden_states, attention_mask, wq, bq, wk, bk, wv, bv, wo, bo,
          ln_gamma, ln_beta):
    """Returns (flags, in_maps)."""
    bf16 = ml_dtypes.bfloat16
    hs = np.asarray(hidden_states, dtype=np.float32)
    mask = np.asarray(attention_mask)
    wq_ = np.asarray(wq, dtype=np.float32).astype(bf16)
    wk_ = np.asarray(wk, dtype=np.float32).astype(bf16)
    wv_ = np.asarray(wv, dtype=np.float32).astype(bf16)
    wo_ = np.asarray(wo, dtype=np.float32).astype(bf16)
    bq_ = np.asarray(bq, dtype=np.float32)
    bk_ = np.asarray(bk, dtype=np.float32)
    bv_ = np.asarray(bv, dtype=np.float32)
    bo_ = np.asarray(bo, dtype=np.float32)
    g_ = np.asarray(ln_gamma, dtype=np.float32)
    be_ = np.asarray(ln_beta, dtype=np.float32)

    use_bv = bool(np.any(bv_ != 0.0))
    use_mask = bool(np.any(mask != 1))
    use_gamma = bool(np.any(g_ != 1.0))
    use_beta = bool(np.any(be_ != 0.0))
    flags = (use_bv, use_mask, use_gamma, use_beta)

    bqr = np.ascontiguousarray(bq_.reshape(DC, P).T)
    bkr = np.ascontiguousarray(bk_.reshape(DC, P).T)

    in_maps = []
    hsT_b = {}
    for b in range(B):
        hsT_b[b] = np.ascontiguousarray(hs[b].T).astype(bf16)
    for c in range(NCORES):
        b, qh = c // 2, c % 2
        q0 = qh * QL
        m = {
            "hsT": hsT_b[b],
            "hsqT": np.ascontiguousarray(hsT_b[b][:, q0:q0 + QL]),
            "res": hs[b, q0:q0 + QL] + bo_,
            "wq": wq_, "wk": wk_, "wv": wv_, "wo": wo_,
            "bqr": bqr, "bkr": bkr,
        }
        if use_bv:
            m["bvrep"] = np.ascontiguousarray(
                np.broadcast_to(bv_, (P, D)).astype(np.float32))
        if use_mask:
            mb = (1.0 - mask[b].astype(np.float32)) * (-10000.0 * 32.0)
            m["mrep"] = np.ascontiguousarray(np.broadcast_to(mb, (P, S)))
            m["mcol"] = np.ascontiguousarray(mb.reshape(DC, P).T)
        if use_gamma:
            m["grep"] = np.ascontiguousarray(np.broadcast_to(g_, (P, D)))
        if use_beta:
            m["brep"] = np.ascontiguousarray(np.broadcast_to(be_, (P, D)))
        in_maps.append(m)
    return flags, in_maps


def kernel(**inputs):
    from concourse.bass_utils import run_bass_kernel_spmd

    flags, in_maps = _prep(**inputs)
    nc = _get_nc(flags)
    r = run_bass_kernel_spmd(nc, in_maps, core_ids=list(range(NCORES)))

    out = np.empty((B, S, D), np.float32)
    probs = np.empty((B, H, S, S), np.float32)
    for c in range(NCORES):
        b, qh = c // 2, c % 2
        q0 = qh * QL
        out[b, q0:q0 + QL] = r.results[c]["out"]
        probs[b, :, q0:q0 + QL, :] = r.results[c]["probs"]
    return out, probs
